# revision 8
# baseline (speedup 1.0000x reference)
"""Trainium2 Bass kernel for nn_MHAAttention (LayerNorm2d + MHA w/ rel-pos bias + residual).

Sharding: data-parallel over batch - 8 batch elements, one per NeuronCore.
No collectives needed.

v3 (bf16 + row-packed head pairs):
  all matmuls bf16 (single-pass PE). Heads processed in pairs (2p, 2p+1):
  head A lives at array rows 0-63, head B at rows 64-127, so their K=64
  score matmuls execute CONCURRENTLY (different row groups + PSUM banks).
  The rel-pos bias is accumulated into PSUM by identity matmuls, split into
  two K=64 halves so each half of head A pairs with the opposite half of
  head B (again different row groups + banks -> concurrent).
  LN rsqrt = exp(-0.5*ln(var+eps)) so one ACT table set serves the kernel.
  attn@V for head A is interleaved jt-by-jt with the score pipeline; head B
  runs from its kept aT tiles afterward. Softmax 1/Z via ln -> K=1 ones
  matmul replication -> exp(-x) on ScalarE.
  Projection: per-ct chains over heads with both query chunks sharing each
  weight load; result staged through a work tile and DMA'd out per chunk.
"""

import sys

for _p in ("/opt/trn_rl_repo",):
    if _p not in sys.path:
        sys.path.insert(0, _p)

from contextlib import ExitStack

import numpy as np
import ml_dtypes

import concourse.bass as bass
import concourse.mybir as mybir
import concourse.tile as tile
from concourse.bass_utils import run_bass_kernel_spmd

F32 = mybir.dt.float32
BF16 = mybir.dt.bfloat16
F16 = mybir.dt.float16
AF = mybir.ActivationFunctionType
OP = mybir.AluOpType

B = 8
CH = 512
H = W = 32
NT = H * W          # 1024 tokens
HEADS = 8
HD = 64
EPS = 1e-6
P = 128
CT = CH // P        # 4 channel tiles
TT = NT // P        # 8 token tiles
IC = NT // 512      # 2 free-dim chunks of 512
STRIP_W = 60 * 32   # 1920
VW = 66             # per-head v stride: [v(64) | 1 | pad]


def _build_strips(rel: np.ndarray) -> np.ndarray:
    """(3969, 8) rel table -> (8, 128, 1920) bias strips.

    strip[h, 32*jh_l + jw, 32*g + iw] = T_h[g - jh_l + 3, iw - jw + 31]
    where T_h = rel[:, h].reshape(63, 63).
    bias.T block for key-tile jt is then strip[:, (28-4*jt)*32 : +1024].
    """
    T = rel.reshape(63, 63, HEADS)  # [a, b, h]
    jh_l = np.arange(4)[:, None, None, None]
    jw = np.arange(32)[None, :, None, None]
    g = np.arange(60)[None, None, :, None]
    iw = np.arange(32)[None, None, None, :]
    a = g - jh_l + 3          # in [0,62]
    b = iw - jw + 31          # in [0,62]
    a_b, b_b = np.broadcast_arrays(a, b)
    out = T[a_b, b_b, :]      # (4, 32, 60, 32, 8)
    out = np.ascontiguousarray(np.moveaxis(out, -1, 0)).reshape(HEADS, 128, STRIP_W)
    return out


def _build_nc() -> bass.Bass:
    nc = bass.Bass()

    x_d = nc.declare_dram_parameter("x", [CH, NT], F32, isOutput=False)
    xb_d = nc.declare_dram_parameter("xb", [CH, NT], BF16, isOutput=False)
    wqT_d = nc.declare_dram_parameter("wqT", [CH, CH], BF16, isOutput=False)
    wkT_d = nc.declare_dram_parameter("wkT", [CH, CH], BF16, isOutput=False)
    wvT_d = nc.declare_dram_parameter("wvT", [CH, CH], BF16, isOutput=False)
    wpP_d = nc.declare_dram_parameter("wpP", [HD, HEADS, CH], BF16, isOutput=False)
    bqk_d = nc.declare_dram_parameter("bqk", [2, CH], F32, isOutput=False)
    brow_d = nc.declare_dram_parameter("brow", [2, CH], BF16, isOutput=False)
    strips_d = nc.declare_dram_parameter("strips", [HEADS, P, STRIP_W], BF16,
                                         isOutput=False)
    ident_d = nc.declare_dram_parameter("ident", [P, P], BF16, isOutput=False)
    y_d = nc.declare_dram_parameter("y", [CH, NT], F32, isOutput=True)

    with tile.TileContext(nc) as tc, ExitStack() as ctx:
        singles = ctx.enter_context(tc.tile_pool(name="singles", bufs=1))
        work = ctx.enter_context(tc.tile_pool(name="work", bufs=4))
        strip_pool = ctx.enter_context(tc.tile_pool(name="strip_pool", bufs=4))
        at_pool = ctx.enter_context(tc.tile_pool(name="at_pool", bufs=24))
        # PSUM budget (8 banks): psA (128,1024)x2bufs = 4 banks (LN stats +
        # scores); psB (128,512)x2 = 2 banks (qkv/proj/zrep); ps_o 2 banks.
        psA = ctx.enter_context(tc.tile_pool(name="psA", bufs=2, space="PSUM"))
        psB = ctx.enter_context(tc.tile_pool(name="psB", bufs=2, space="PSUM"))
        ps_o = ctx.enter_context(tc.tile_pool(name="ps_o", bufs=1, space="PSUM"))

        # ---------- persistent SBUF ----------
        x_sb = singles.tile([P, CT, NT], F32)        # residual source
        xb_sb = singles.tile([P, CT, NT], BF16)      # bf16 x for stats
        xn_sb = singles.tile([P, CT, NT], BF16)      # LN output
        qT_sb = singles.tile([P, CT, NT], BF16)      # (d part, t free)
        kT_sb = singles.tile([P, CT, NT], BF16)
        v_sb = singles.tile([P, TT, HEADS * VW], BF16)
        oTn_sb = singles.tile([HD, HEADS, NT], BF16)  # normalized per-head oT

        wq_sb = singles.tile([P, CT, CH], BF16)
        wk_sb = singles.tile([P, CT, CH], BF16)
        wv_sb = singles.tile([P, CT, CH], BF16)
        wpP_sb = singles.tile([HD, HEADS, CH], BF16)
        bqk_sb = singles.tile([P, 2, CT], F32)       # per-partition bias for q,k
        brow_sb = singles.tile([1, 2, CH], BF16)     # bv_eff, bp rows
        ident_sb = singles.tile([P, P], BF16)
        ones_mb = singles.tile([P, P], BF16)         # bf16 ones (LN stats lhsT)
        ones_rb = singles.tile([1, 512], BF16)       # bf16 ones row
        ones16 = singles.tile([HD + 1, HD], F16)     # f16 ones (zrep lhsT, row 64)
        lnz_sb = singles.tile([HD + 1, NT], F16)     # ln(Z) row at partition 64

        mu_b = singles.tile([P, NT], BF16)
        rs_b = singles.tile([P, NT], BF16)
        m2_f = singles.tile([P, NT], F32)
        ve_f = singles.tile([P, NT], F32)

        nc.vector.memset(ones_mb[:], 1.0)
        nc.vector.memset(ones_rb[:], 1.0)
        nc.vector.memset(ones16[:], 1.0)
        nc.sync.dma_start(ident_sb[:], ident_d[:])
        nc.sync.dma_start(bqk_sb[:], bqk_d.rearrange("i (o p) -> p i o", p=P))
        nc.sync.dma_start(brow_sb[:], brow_d[None, :, :])
        # per-ct x chunks so LN stats can start on the first chunk
        xb_r = xb_d.rearrange("(ct p) t -> p ct t", p=P)
        for ct in range(CT):
            nc.sync.dma_start(xb_sb[:, ct], xb_r[:, ct])
        nc.sync.dma_start(wq_sb[:], wqT_d.rearrange("(ck p) d -> p ck d", p=P))
        nc.sync.dma_start(wk_sb[:], wkT_d.rearrange("(ck p) d -> p ck d", p=P))
        nc.sync.dma_start(wv_sb[:], wvT_d.rearrange("(ck p) d -> p ck d", p=P))
        nc.sync.dma_start(wpP_sb[:], wpP_d[:])

        # ones columns of v
        v_view = v_sb[:].rearrange("p tt (h w) -> p tt h w", w=VW)
        nc.vector.memset(v_view[:, :, :, HD : HD + 1], 1.0)

        # ---------- phase 1: LayerNorm ----------
        with tc.tile_pool(name="ln_pool", bufs=2) as lnp:
            sum_ps = psA.tile([P, NT], F32, tag="big")
            sq_ps = psA.tile([P, NT], F32, tag="big")
            for ct in range(CT):
                x2 = lnp.tile([P, NT], BF16, name=f"x2_{ct}", tag="x2")
                nc.vector.tensor_tensor(out=x2[:], in0=xb_sb[:, ct],
                                        in1=xb_sb[:, ct], op=OP.mult)
                for ic in range(IC):
                    sl = slice(ic * 512, ic * 512 + 512)
                    nc.tensor.matmul(sum_ps[:, sl], lhsT=ones_mb[:],
                                     rhs=xb_sb[:, ct, sl],
                                     start=(ct == 0), stop=(ct == CT - 1))
                    nc.tensor.matmul(sq_ps[:, sl], lhsT=ones_mb[:], rhs=x2[:, sl],
                                     start=(ct == 0), stop=(ct == CT - 1))

            # mu (bf16 for the apply; bf16 is fine inside 512*mu^2 too)
            nc.scalar.activation(out=mu_b[:], in_=sum_ps[:], func=AF.Copy,
                                 scale=1.0 / CH)
            # 512*mu^2 ; (var+eps)*512 = (sq + 512*eps) - 512*mu^2
            nc.vector.tensor_tensor(out=m2_f[:], in0=mu_b[:], in1=sum_ps[:],
                                    op=OP.mult)
            nc.vector.scalar_tensor_tensor(out=ve_f[:], in0=sq_ps[:],
                                           scalar=float(CH * EPS), in1=m2_f[:],
                                           op0=OP.add, op1=OP.subtract)
            # rs = rsqrt(var+eps) = exp(-0.5*ln(var+eps)); keeps ACT on the
            # natural_log_exp table set for the entire kernel
            nc.scalar.activation(out=ve_f[:], in_=ve_f[:], func=AF.Ln,
                                 scale=1.0 / CH)
            nc.scalar.activation(out=rs_b[:], in_=ve_f[:], func=AF.Exp,
                                 scale=-0.5)

            for ct in range(CT):
                nc.vector.tensor_tensor(out=xn_sb[:, ct], in0=xb_sb[:, ct],
                                        in1=mu_b[:], op=OP.subtract)
                nc.vector.tensor_tensor(out=xn_sb[:, ct], in0=xn_sb[:, ct],
                                        in1=rs_b[:], op=OP.mult)

        # prefetch strips for the first head pair; residual x late (proj-only)
        strip_tiles = {}
        for h in (0, 1):
            st = strip_pool.tile([P, STRIP_W], BF16, name=f"strip{h}", tag="strip")
            nc.sync.dma_start(st[:], strips_d[h])
            strip_tiles[h] = st
        x_r = x_d.rearrange("(ct p) t -> p ct t", p=P)
        for ct in range(CT):
            nc.sync.dma_start(x_sb[:, ct], x_r[:, ct])

        # ---------- phase 2: Q, K, V projections ----------
        for dt in range(CT):
            dsl = slice(dt * P, dt * P + P)
            for ic in range(IC):
                sl = slice(ic * 512, ic * 512 + 512)
                q_ps = psB.tile([P, 512], F32, tag="small")
                for ck in range(CT):
                    nc.tensor.matmul(q_ps[:], lhsT=wq_sb[:, ck, dsl],
                                     rhs=xn_sb[:, ck, sl],
                                     start=(ck == 0), stop=(ck == CT - 1))
                nc.vector.tensor_scalar_add(out=qT_sb[:, dt, sl], in0=q_ps[:],
                                            scalar1=bqk_sb[:, 0, dt : dt + 1])
                k_ps = psB.tile([P, 512], F32, tag="small")
                for ck in range(CT):
                    nc.tensor.matmul(k_ps[:], lhsT=wk_sb[:, ck, dsl],
                                     rhs=xn_sb[:, ck, sl],
                                     start=(ck == 0), stop=(ck == CT - 1))
                nc.vector.tensor_scalar_add(out=kT_sb[:, dt, sl], in0=k_ps[:],
                                            scalar1=bqk_sb[:, 1, dt : dt + 1])

        for tt in range(TT):
            tsl = slice(tt * P, tt * P + P)
            v_ps = psB.tile([P, 512], F32, tag="small")
            for ck in range(CT):
                nc.tensor.matmul(v_ps[:], lhsT=xn_sb[:, ck, tsl],
                                 rhs=wv_sb[:, ck, :],
                                 start=(ck == 0), stop=False)
            nc.tensor.matmul(v_ps[:], lhsT=ones_rb[:, :P], rhs=brow_sb[:, 0, :],
                             start=False, stop=True)
            nc.vector.tensor_copy(
                out=v_view[:, tt, :, 0:HD],
                in_=v_ps[:].rearrange("p (h w) -> p h w", w=HD))

        # ---------- phase 3: attention, head pairs (A rows 0-63, B rows 64-127) --
        iA = ident_sb[0:HD, :]        # I[0:64]:  out[j]=strip[j]  j<64
        iB = ident_sb[HD:P, :]        # I[64:128]: out[j]=strip[j] j>=64

        def zpath(h, o_ps):
            """ln(Z) -> replicate via K=1 matmul -> exp(-x) -> oTn."""
            nc.scalar.activation(out=lnz_sb[HD : HD + 1, :],
                                 in_=o_ps[HD : HD + 1, :], func=AF.Ln)
            for ic in range(IC):
                sl = slice(ic * 512, ic * 512 + 512)
                zl_ps = psB.tile([P, 512], F32, tag="small")
                nc.tensor.matmul(zl_ps[:HD, :], lhsT=ones16[HD : HD + 1, :],
                                 rhs=lnz_sb[HD : HD + 1, sl],
                                 start=True, stop=True)
                zrep = work.tile([HD, 512], F32, tag="zrep")
                nc.scalar.activation(out=zrep[:], in_=zl_ps[:HD, :], func=AF.Exp,
                                     scale=-1.0)
                nc.vector.tensor_tensor(out=oTn_sb[:, h, sl], in0=o_ps[:HD, sl],
                                        in1=zrep[:], op=OP.mult)

        for pr in range(HEADS // 2):
            hA, hB = 2 * pr, 2 * pr + 1
            stripA = strip_tiles.pop(hA)
            stripB = strip_tiles.pop(hB)
            if pr < HEADS // 2 - 1:
                for h in (hA + 2, hB + 2):
                    st = strip_pool.tile([P, STRIP_W], BF16, name=f"strip{h}",
                                         tag="strip")
                    nc.sync.dma_start(st[:], strips_d[h])
                    strip_tiles[h] = st

            atA, atB = [], []
            for jt in range(TT):
                sA = psA.tile([P, NT], F32, tag="big")
                sB = psA.tile([P, NT], F32, tag="big")
                off = (28 - 4 * jt) * 32
                jsl = slice(jt * P, jt * P + P)
                for ic in range(IC):
                    sl = slice(ic * 512, ic * 512 + 512)
                    so = slice(off + ic * 512, off + ic * 512 + 512)
                    nc.tensor.matmul(sA[:, sl], lhsT=ident_sb[:],
                                     rhs=stripA[:, so], start=True, stop=False)
                    nc.tensor.matmul(sB[:, sl], lhsT=ident_sb[:],
                                     rhs=stripB[:, so], start=True, stop=False)
                    # scores: head A rows 0-63, head B rows 64-127, concurrent
                    nc.tensor.matmul(sA[:, sl], lhsT=kT_sb[0:HD, pr, jsl],
                                     rhs=qT_sb[0:HD, pr, sl],
                                     start=False, stop=True)
                    nc.tensor.matmul(sB[:, sl], lhsT=kT_sb[HD:P, pr, jsl],
                                     rhs=qT_sb[HD:P, pr, sl],
                                     start=False, stop=True)
                aT = at_pool.tile([P, NT], BF16, name=f"aT_{hA}_{jt}", tag="aT")
                nc.scalar.activation(out=aT[:], in_=sA[:], func=AF.Exp)
                atA.append(aT)
                bT = at_pool.tile([P, NT], BF16, name=f"aT_{hB}_{jt}", tag="aT")
                nc.scalar.activation(out=bT[:], in_=sB[:], func=AF.Exp)
                atB.append(bT)

            for hh, at_tiles in ((hA, atA), (hB, atB)):
                o_ps = ps_o.tile([HD + 1, NT], F32, tag="o",
                                 name=f"o_ps_{hh}")
                for jt in range(TT):
                    for ic in range(IC):
                        sl = slice(ic * 512, ic * 512 + 512)
                        nc.tensor.matmul(
                            o_ps[:, sl],
                            lhsT=v_sb[:, jt, hh * VW : hh * VW + HD + 1],
                            rhs=at_tiles[jt][:, sl],
                            start=(jt == 0), stop=(jt == TT - 1))
                zpath(hh, o_ps)

        # ---------- phase 4: output projection + residual ----------
        for ct in range(CT):
            csl = slice(ct * P, ct * P + P)
            y_ps = [psB.tile([P, 512], F32, tag="small", name=f"y_ps_{ct}_{i}")
                    for i in range(IC)]
            for h in range(HEADS):
                for icc in range(IC):
                    sl = slice(icc * 512, icc * 512 + 512)
                    nc.tensor.matmul(y_ps[icc][:], lhsT=wpP_sb[:, h, csl],
                                     rhs=oTn_sb[:, h, sl],
                                     start=(h == 0), stop=False)
            for icc in range(IC):
                sl = slice(icc * 512, icc * 512 + 512)
                nc.tensor.matmul(y_ps[icc][:], lhsT=brow_sb[:, 1, csl],
                                 rhs=ones_rb[:, :512],
                                 start=False, stop=True)
                yw = work.tile([P, 512], F32, tag="yw")
                nc.vector.tensor_tensor(out=yw[:], in0=y_ps[icc][:],
                                        in1=x_sb[:, ct, sl], op=OP.add)
                nc.sync.dma_start(y_d[csl, sl], yw[:])

    return nc


def _legalize_waits(nc, max_waits: int = 1):
    """Split multi-wait instructions into preceding same-engine NoOps.

    The TPB instruction encoding carries a single sync-wait slot and this
    walrus build refuses to legalize ("Too many sync wait commands"), so do
    it here: engines execute their queue in order, so a NoOp carrying one of
    the waits delays everything after it on that engine identically.
    """
    import orjson

    data = orjson.loads(mybir.module_to_json_bytes(nc.m))
    ctr = [0]

    def fix_block(block):
        out = []
        for inst in block.get("instructions", []):
            si = inst.get("sync_info") or {}
            waits = si.get("on_wait") or []
            if len(waits) > max_waits:
                for w in waits[max_waits:]:
                    ctr[0] += 1
                    nop = {
                        "name": f"I-WS{ctr[0]}",
                        "opcode": "NoOp",
                        "engine": inst["engine"],
                        "ins": [],
                        "outs": [],
                        "sync_info": {"on_wait": [w], "on_update": []},
                    }
                    if "debug" in inst:
                        nop["debug"] = inst["debug"]
                    out.append(nop)
                si = dict(si)
                si["on_wait"] = waits[:max_waits]
                inst["sync_info"] = si
            out.append(inst)
        block["instructions"] = out
        for b in block.get("blocks", []):
            fix_block(b)

    for fn in data["functions"]:
        for b in fn.get("blocks", []):
            fix_block(b)
    nc.m = mybir.module_from_json_bytes(orjson.dumps(data))
    return nc


_NC = None

BF = ml_dtypes.bfloat16


def _host_prep(x, norm_w, norm_b, wq, bq, wk, bk, wv, bv, wp, bp, rel):
    scale = HD ** -0.5
    # fold LN affine + score scale into the projection weights (exact algebra)
    wq_eff = (wq * norm_w[None, :]) * scale
    bq_eff = (bq + wq @ norm_b) * scale
    wk_eff = wk * norm_w[None, :]
    bk_eff = bk + wk @ norm_b
    wv_eff = wv * norm_w[None, :]
    bv_eff = bv + wv @ norm_b

    wqT = np.ascontiguousarray(wq_eff.T).astype(BF)
    wkT = np.ascontiguousarray(wk_eff.T).astype(BF)
    wvT = np.ascontiguousarray(wv_eff.T).astype(BF)
    # wp permuted so each head's 64 input rows sit at partitions 0..63
    wpP = np.ascontiguousarray(
        wp.T.reshape(HEADS, HD, CH).transpose(1, 0, 2)).astype(BF)

    bqk = np.stack([bq_eff, bk_eff]).astype(np.float32)
    brow = np.stack([bv_eff, bp]).astype(BF)
    strips = _build_strips(np.asarray(rel, np.float32)).astype(BF)
    ident = np.eye(P, dtype=BF)

    shared = {
        "wqT": wqT, "wkT": wkT, "wvT": wvT, "wpP": wpP,
        "bqk": bqk, "brow": brow, "strips": strips, "ident": ident,
    }
    in_maps = []
    for b in range(B):
        m = dict(shared)
        xf = np.ascontiguousarray(x[b].reshape(CH, NT)).astype(np.float32)
        m["x"] = xf
        m["xb"] = xf.astype(BF)
        in_maps.append(m)
    return in_maps


def kernel(**inputs):
    global _NC
    if _NC is None:
        _NC = _legalize_waits(_build_nc())
    in_maps = _host_prep(**{k: np.asarray(v) for k, v in inputs.items()})
    res = run_bass_kernel_spmd(_NC, in_maps, list(range(B)))
    out = np.stack([res.results[b]["y"].reshape(CH, H, W) for b in range(B)])
    return out.astype(np.float32)


if __name__ == "__main__":
    nc = _build_nc()
    print("built OK")


# revision 10
# speedup vs baseline: 1.1016x; 1.1016x over previous
"""Trainium2 Bass kernel for nn_MHAAttention (LayerNorm2d + MHA w/ rel-pos bias + residual).

Sharding: data-parallel over batch - 8 batch elements, one per NeuronCore.
No collectives needed.

v3 (bf16 + row-packed head pairs):
  all matmuls bf16 (single-pass PE). Heads processed in pairs (2p, 2p+1):
  head A lives at array rows 0-63, head B at rows 64-127, so their K=64
  score matmuls execute CONCURRENTLY (different row groups + PSUM banks).
  The rel-pos bias is accumulated into PSUM by identity matmuls, split into
  two K=64 halves so each half of head A pairs with the opposite half of
  head B (again different row groups + banks -> concurrent).
  LN rsqrt = exp(-0.5*ln(var+eps)) so one ACT table set serves the kernel.
  attn@V for head A is interleaved jt-by-jt with the score pipeline; head B
  runs from its kept aT tiles afterward. Softmax 1/Z via ln -> K=1 ones
  matmul replication -> exp(-x) on ScalarE.
  Projection: per-ct chains over heads with both query chunks sharing each
  weight load; result staged through a work tile and DMA'd out per chunk.
"""

import sys

for _p in ("/opt/trn_rl_repo",):
    if _p not in sys.path:
        sys.path.insert(0, _p)

from contextlib import ExitStack

import numpy as np
import ml_dtypes

import concourse.bass as bass
import concourse.mybir as mybir
import concourse.tile as tile
from concourse.bass_utils import run_bass_kernel_spmd

F32 = mybir.dt.float32
BF16 = mybir.dt.bfloat16
F16 = mybir.dt.float16
AF = mybir.ActivationFunctionType
OP = mybir.AluOpType

B = 8
CH = 512
H = W = 32
NT = H * W          # 1024 tokens
HEADS = 8
HD = 64
EPS = 1e-6
P = 128
CT = CH // P        # 4 channel tiles
TT = NT // P        # 8 token tiles
IC = NT // 512      # 2 free-dim chunks of 512
STRIP_W = 60 * 32   # 1920
VW = 66             # per-head v stride: [v(64) | 1 | pad]


def _build_strips(rel: np.ndarray) -> np.ndarray:
    """(3969, 8) rel table -> (8, 128, 1920) bias strips.

    strip[h, 32*jh_l + jw, 32*g + iw] = T_h[g - jh_l + 3, iw - jw + 31]
    where T_h = rel[:, h].reshape(63, 63).
    bias.T block for key-tile jt is then strip[:, (28-4*jt)*32 : +1024].
    """
    T = rel.reshape(63, 63, HEADS)  # [a, b, h]
    jh_l = np.arange(4)[:, None, None, None]
    jw = np.arange(32)[None, :, None, None]
    g = np.arange(60)[None, None, :, None]
    iw = np.arange(32)[None, None, None, :]
    a = g - jh_l + 3          # in [0,62]
    b = iw - jw + 31          # in [0,62]
    a_b, b_b = np.broadcast_arrays(a, b)
    out = T[a_b, b_b, :]      # (4, 32, 60, 32, 8)
    out = np.ascontiguousarray(np.moveaxis(out, -1, 0)).reshape(HEADS, 128, STRIP_W)
    return out


def _build_nc() -> bass.Bass:
    nc = bass.Bass()

    x_d = nc.declare_dram_parameter("x", [CH, NT], F32, isOutput=False)
    xb_d = nc.declare_dram_parameter("xb", [CH, NT], BF16, isOutput=False)
    wqT_d = nc.declare_dram_parameter("wqT", [CH, CH], BF16, isOutput=False)
    wkT_d = nc.declare_dram_parameter("wkT", [CH, CH], BF16, isOutput=False)
    wvT_d = nc.declare_dram_parameter("wvT", [CH, CH], BF16, isOutput=False)
    wpP_d = nc.declare_dram_parameter("wpP", [HD, HEADS, CH], BF16, isOutput=False)
    bqk_d = nc.declare_dram_parameter("bqk", [2, CH], F32, isOutput=False)
    brow_d = nc.declare_dram_parameter("brow", [2, CH], BF16, isOutput=False)
    strips_d = nc.declare_dram_parameter("strips", [HEADS, P, STRIP_W], BF16,
                                         isOutput=False)
    y_d = nc.declare_dram_parameter("y", [CH, NT], F32, isOutput=True)

    with tile.TileContext(nc) as tc, ExitStack() as ctx:
        singles = ctx.enter_context(tc.tile_pool(name="singles", bufs=1))
        work = ctx.enter_context(tc.tile_pool(name="work", bufs=4))
        es_pool = ctx.enter_context(tc.tile_pool(name="es_pool", bufs=3))
        strip_pool = ctx.enter_context(tc.tile_pool(name="strip_pool", bufs=4))
        at_pool = ctx.enter_context(tc.tile_pool(name="at_pool", bufs=24))
        # PSUM budget (8 banks): psA (128,1024)x2bufs = 4 banks (LN stats +
        # scores); psB (128,512)x2 = 2 banks (qkv/proj/zrep); ps_o 2 banks.
        psA = ctx.enter_context(tc.tile_pool(name="psA", bufs=2, space="PSUM"))
        psB = ctx.enter_context(tc.tile_pool(name="psB", bufs=2, space="PSUM"))
        ps_o = ctx.enter_context(tc.tile_pool(name="ps_o", bufs=1, space="PSUM"))

        # ---------- persistent SBUF ----------
        x_sb = singles.tile([P, CT, NT], F32)        # residual source
        xb_sb = singles.tile([P, CT, NT], BF16)      # bf16 x for stats
        xn_sb = singles.tile([P, CT, NT], BF16)      # LN output
        qT_sb = singles.tile([P, CT, NT], BF16)      # (d part, t free)
        kT_sb = singles.tile([P, CT, NT], BF16)
        v_sb = singles.tile([P, TT, HEADS * VW], BF16)
        oTn_sb = singles.tile([HD, HEADS, NT], BF16)  # normalized per-head oT

        wq_sb = singles.tile([P, CT, CH], BF16)
        wk_sb = singles.tile([P, CT, CH], BF16)
        wv_sb = singles.tile([P, CT, CH], BF16)
        wpP_sb = singles.tile([HD, HEADS, CH], BF16)
        bqk_sb = singles.tile([P, 2, CT], F32)       # per-partition bias for q,k
        brow_sb = singles.tile([1, 2, CH], BF16)     # bv_eff, bp rows
        ones_mb = singles.tile([P, P], BF16)         # bf16 ones (LN stats lhsT)
        ones_rb = singles.tile([1, 512], BF16)       # bf16 ones row
        ones16 = singles.tile([HD + 1, HD], F16)     # f16 ones (zrep lhsT, row 64)
        lnz_sb = singles.tile([HD + 1, NT], F16)     # ln(Z) row at partition 64

        mu_b = singles.tile([P, NT], BF16)
        rs_b = singles.tile([P, NT], BF16)
        m2_f = singles.tile([P, NT], F32)
        ve_f = singles.tile([P, NT], F32)

        nc.vector.memset(ones_mb[:], 1.0)
        nc.vector.memset(ones_rb[:], 1.0)
        nc.vector.memset(ones16[:], 1.0)
        nc.sync.dma_start(bqk_sb[:], bqk_d.rearrange("i (o p) -> p i o", p=P))
        nc.sync.dma_start(brow_sb[:], brow_d[None, :, :])
        # per-ct x chunks so LN stats can start on the first chunk
        xb_r = xb_d.rearrange("(ct p) t -> p ct t", p=P)
        for ct in range(CT):
            nc.sync.dma_start(xb_sb[:, ct], xb_r[:, ct])
        nc.sync.dma_start(wq_sb[:], wqT_d.rearrange("(ck p) d -> p ck d", p=P))
        nc.sync.dma_start(wk_sb[:], wkT_d.rearrange("(ck p) d -> p ck d", p=P))
        nc.sync.dma_start(wv_sb[:], wvT_d.rearrange("(ck p) d -> p ck d", p=P))
        nc.sync.dma_start(wpP_sb[:], wpP_d[:])

        # ones columns of v
        v_view = v_sb[:].rearrange("p tt (h w) -> p tt h w", w=VW)
        nc.vector.memset(v_view[:, :, :, HD : HD + 1], 1.0)

        # ---------- phase 1: LayerNorm ----------
        with tc.tile_pool(name="ln_pool", bufs=2) as lnp:
            sum_ps = psA.tile([P, NT], F32, tag="big")
            sq_ps = psA.tile([P, NT], F32, tag="big")
            for ct in range(CT):
                x2 = lnp.tile([P, NT], BF16, name=f"x2_{ct}", tag="x2")
                nc.vector.tensor_tensor(out=x2[:], in0=xb_sb[:, ct],
                                        in1=xb_sb[:, ct], op=OP.mult)
                for ic in range(IC):
                    sl = slice(ic * 512, ic * 512 + 512)
                    nc.tensor.matmul(sum_ps[:, sl], lhsT=ones_mb[:],
                                     rhs=xb_sb[:, ct, sl],
                                     start=(ct == 0), stop=(ct == CT - 1))
                    nc.tensor.matmul(sq_ps[:, sl], lhsT=ones_mb[:], rhs=x2[:, sl],
                                     start=(ct == 0), stop=(ct == CT - 1))

            # mu (bf16 for the apply; bf16 is fine inside 512*mu^2 too)
            nc.scalar.activation(out=mu_b[:], in_=sum_ps[:], func=AF.Copy,
                                 scale=1.0 / CH)
            # 512*mu^2 ; (var+eps)*512 = (sq + 512*eps) - 512*mu^2
            nc.vector.tensor_tensor(out=m2_f[:], in0=mu_b[:], in1=sum_ps[:],
                                    op=OP.mult)
            nc.vector.scalar_tensor_tensor(out=ve_f[:], in0=sq_ps[:],
                                           scalar=float(CH * EPS), in1=m2_f[:],
                                           op0=OP.add, op1=OP.subtract)
            # rs = rsqrt(var+eps) = exp(-0.5*ln(var+eps)); keeps ACT on the
            # natural_log_exp table set for the entire kernel
            nc.scalar.activation(out=ve_f[:], in_=ve_f[:], func=AF.Ln,
                                 scale=1.0 / CH)
            nc.scalar.activation(out=rs_b[:], in_=ve_f[:], func=AF.Exp,
                                 scale=-0.5)

            for ct in range(CT):
                nc.vector.tensor_tensor(out=xn_sb[:, ct], in0=xb_sb[:, ct],
                                        in1=mu_b[:], op=OP.subtract)
                nc.vector.tensor_tensor(out=xn_sb[:, ct], in0=xn_sb[:, ct],
                                        in1=rs_b[:], op=OP.mult)

        # prefetch strips for the first head pair; residual x late (proj-only)
        strip_tiles = {}
        for h in (0, 1):
            st = strip_pool.tile([P, STRIP_W], BF16, name=f"strip{h}", tag="strip")
            nc.sync.dma_start(st[:], strips_d[h])
            strip_tiles[h] = st
        x_r = x_d.rearrange("(ct p) t -> p ct t", p=P)
        for ct in range(CT):
            nc.sync.dma_start(x_sb[:, ct], x_r[:, ct])

        # ---------- phase 2: Q, K, V projections ----------
        for dt in range(CT):
            dsl = slice(dt * P, dt * P + P)
            for ic in range(IC):
                sl = slice(ic * 512, ic * 512 + 512)
                q_ps = psB.tile([P, 512], F32, tag="small")
                for ck in range(CT):
                    nc.tensor.matmul(q_ps[:], lhsT=wq_sb[:, ck, dsl],
                                     rhs=xn_sb[:, ck, sl],
                                     start=(ck == 0), stop=(ck == CT - 1))
                nc.vector.tensor_scalar_add(out=qT_sb[:, dt, sl], in0=q_ps[:],
                                            scalar1=bqk_sb[:, 0, dt : dt + 1])
                k_ps = psB.tile([P, 512], F32, tag="small")
                for ck in range(CT):
                    nc.tensor.matmul(k_ps[:], lhsT=wk_sb[:, ck, dsl],
                                     rhs=xn_sb[:, ck, sl],
                                     start=(ck == 0), stop=(ck == CT - 1))
                nc.vector.tensor_scalar_add(out=kT_sb[:, dt, sl], in0=k_ps[:],
                                            scalar1=bqk_sb[:, 1, dt : dt + 1])

        for tt in range(TT):
            tsl = slice(tt * P, tt * P + P)
            v_ps = psB.tile([P, 512], F32, tag="small")
            for ck in range(CT):
                nc.tensor.matmul(v_ps[:], lhsT=xn_sb[:, ck, tsl],
                                 rhs=wv_sb[:, ck, :],
                                 start=(ck == 0), stop=False)
            nc.tensor.matmul(v_ps[:], lhsT=ones_rb[:, :P], rhs=brow_sb[:, 0, :],
                             start=False, stop=True)
            nc.vector.tensor_copy(
                out=v_view[:, tt, :, 0:HD],
                in_=v_ps[:].rearrange("p (h w) -> p h w", w=HD))

        # ---------- phase 3: attention, head pairs (A rows 0-63, B rows 64-127) --
        def zpath(h, o_ps):
            """ln(Z) -> replicate via K=1 matmul -> exp(-x) -> oTn."""
            nc.scalar.activation(out=lnz_sb[HD : HD + 1, :],
                                 in_=o_ps[HD : HD + 1, :], func=AF.Ln)
            for ic in range(IC):
                sl = slice(ic * 512, ic * 512 + 512)
                zl_ps = psB.tile([P, 512], F32, tag="small")
                nc.tensor.matmul(zl_ps[:HD, :], lhsT=ones16[HD : HD + 1, :],
                                 rhs=lnz_sb[HD : HD + 1, sl],
                                 start=True, stop=True)
                zrep = work.tile([HD, 512], F32, tag="zrep")
                nc.scalar.activation(out=zrep[:], in_=zl_ps[:HD, :], func=AF.Exp,
                                     scale=-1.0)
                nc.vector.tensor_tensor(out=oTn_sb[:, h, sl], in0=o_ps[:HD, sl],
                                        in1=zrep[:], op=OP.mult)

        for h in range(HEADS):
            dtl = h // 2
            drow = HD * (h % 2)
            strip = strip_tiles.pop(h)
            if h < HEADS - 2:
                st = strip_pool.tile([P, STRIP_W], BF16, name=f"strip{h + 2}",
                                     tag="strip")
                nc.sync.dma_start(st[:], strips_d[h + 2])
                strip_tiles[h + 2] = st

            at_tiles = []
            for jt in range(TT):
                s_ps = psA.tile([P, NT], F32, tag="big")
                off = (28 - 4 * jt) * 32
                for ic in range(IC):
                    sl = slice(ic * 512, ic * 512 + 512)
                    nc.tensor.matmul(
                        s_ps[:, sl],
                        lhsT=kT_sb[drow : drow + HD, dtl, jt * P : jt * P + P],
                        rhs=qT_sb[drow : drow + HD, dtl, sl],
                        start=True, stop=True)
                # exp(s) on ScalarE, then * exp(bias) on DVE (bf16 2x mode):
                # exp(s + b) = exp(s) * exp(b), strips hold exp(b) host-side
                eS = es_pool.tile([P, NT], BF16, name=f"eS_{h}_{jt}", tag="eS")
                nc.scalar.activation(out=eS[:], in_=s_ps[:], func=AF.Exp)
                aT = at_pool.tile([P, NT], BF16, name=f"aT_{h}_{jt}", tag="aT")
                nc.vector.tensor_tensor(out=aT[:], in0=eS[:],
                                        in1=strip[:, off : off + NT], op=OP.mult)
                at_tiles.append(aT)

            o_ps = ps_o.tile([HD + 1, NT], F32, tag="o")
            for jt in range(TT):
                for ic in range(IC):
                    sl = slice(ic * 512, ic * 512 + 512)
                    nc.tensor.matmul(
                        o_ps[:, sl],
                        lhsT=v_sb[:, jt, h * VW : h * VW + HD + 1],
                        rhs=at_tiles[jt][:, sl],
                        start=(jt == 0), stop=(jt == TT - 1))
            zpath(h, o_ps)

        # ---------- phase 4: output projection + residual ----------
        for ct in range(CT):
            csl = slice(ct * P, ct * P + P)
            y_ps = [psB.tile([P, 512], F32, tag="small", name=f"y_ps_{ct}_{i}")
                    for i in range(IC)]
            for h in range(HEADS):
                for icc in range(IC):
                    sl = slice(icc * 512, icc * 512 + 512)
                    nc.tensor.matmul(y_ps[icc][:], lhsT=wpP_sb[:, h, csl],
                                     rhs=oTn_sb[:, h, sl],
                                     start=(h == 0), stop=False)
            for icc in range(IC):
                sl = slice(icc * 512, icc * 512 + 512)
                nc.tensor.matmul(y_ps[icc][:], lhsT=brow_sb[:, 1, csl],
                                 rhs=ones_rb[:, :512],
                                 start=False, stop=True)
                yw = work.tile([P, 512], F32, tag="yw")
                nc.vector.tensor_tensor(out=yw[:], in0=y_ps[icc][:],
                                        in1=x_sb[:, ct, sl], op=OP.add)
                nc.sync.dma_start(y_d[csl, sl], yw[:])

    return nc


def _legalize_waits(nc, max_waits: int = 1):
    """Split multi-wait instructions into preceding same-engine NoOps.

    The TPB instruction encoding carries a single sync-wait slot and this
    walrus build refuses to legalize ("Too many sync wait commands"), so do
    it here: engines execute their queue in order, so a NoOp carrying one of
    the waits delays everything after it on that engine identically.
    """
    import orjson

    data = orjson.loads(mybir.module_to_json_bytes(nc.m))
    ctr = [0]

    def fix_block(block):
        out = []
        for inst in block.get("instructions", []):
            si = inst.get("sync_info") or {}
            waits = si.get("on_wait") or []
            if len(waits) > max_waits:
                for w in waits[max_waits:]:
                    ctr[0] += 1
                    nop = {
                        "name": f"I-WS{ctr[0]}",
                        "opcode": "NoOp",
                        "engine": inst["engine"],
                        "ins": [],
                        "outs": [],
                        "sync_info": {"on_wait": [w], "on_update": []},
                    }
                    if "debug" in inst:
                        nop["debug"] = inst["debug"]
                    out.append(nop)
                si = dict(si)
                si["on_wait"] = waits[:max_waits]
                inst["sync_info"] = si
            out.append(inst)
        block["instructions"] = out
        for b in block.get("blocks", []):
            fix_block(b)

    for fn in data["functions"]:
        for b in fn.get("blocks", []):
            fix_block(b)
    nc.m = mybir.module_from_json_bytes(orjson.dumps(data))
    return nc


_NC = None

BF = ml_dtypes.bfloat16


def _host_prep(x, norm_w, norm_b, wq, bq, wk, bk, wv, bv, wp, bp, rel):
    scale = HD ** -0.5
    # fold LN affine + score scale into the projection weights (exact algebra)
    wq_eff = (wq * norm_w[None, :]) * scale
    bq_eff = (bq + wq @ norm_b) * scale
    wk_eff = wk * norm_w[None, :]
    bk_eff = bk + wk @ norm_b
    wv_eff = wv * norm_w[None, :]
    bv_eff = bv + wv @ norm_b

    wqT = np.ascontiguousarray(wq_eff.T).astype(BF)
    wkT = np.ascontiguousarray(wk_eff.T).astype(BF)
    wvT = np.ascontiguousarray(wv_eff.T).astype(BF)
    # wp permuted so each head's 64 input rows sit at partitions 0..63
    wpP = np.ascontiguousarray(
        wp.T.reshape(HEADS, HD, CH).transpose(1, 0, 2)).astype(BF)

    bqk = np.stack([bq_eff, bk_eff]).astype(np.float32)
    brow = np.stack([bv_eff, bp]).astype(BF)
    strips = np.exp(_build_strips(np.asarray(rel, np.float32))).astype(BF)

    shared = {
        "wqT": wqT, "wkT": wkT, "wvT": wvT, "wpP": wpP,
        "bqk": bqk, "brow": brow, "strips": strips,
    }
    in_maps = []
    for b in range(B):
        m = dict(shared)
        xf = np.ascontiguousarray(x[b].reshape(CH, NT)).astype(np.float32)
        m["x"] = xf
        m["xb"] = xf.astype(BF)
        in_maps.append(m)
    return in_maps


def kernel(**inputs):
    global _NC
    if _NC is None:
        _NC = _legalize_waits(_build_nc())
    in_maps = _host_prep(**{k: np.asarray(v) for k, v in inputs.items()})
    res = run_bass_kernel_spmd(_NC, in_maps, list(range(B)))
    out = np.stack([res.results[b]["y"].reshape(CH, H, W) for b in range(B)])
    return out.astype(np.float32)


if __name__ == "__main__":
    nc = _build_nc()
    print("built OK")


# revision 11
# speedup vs baseline: 1.1449x; 1.0394x over previous
"""Trainium2 Bass kernel for nn_MHAAttention (LayerNorm2d + MHA w/ rel-pos bias + residual).

Sharding: data-parallel over batch - 8 batch elements, one per NeuronCore.
No collectives needed.

v3 (bf16 + row-packed head pairs):
  all matmuls bf16 (single-pass PE). Heads processed in pairs (2p, 2p+1):
  head A lives at array rows 0-63, head B at rows 64-127, so their K=64
  score matmuls execute CONCURRENTLY (different row groups + PSUM banks).
  The rel-pos bias is accumulated into PSUM by identity matmuls, split into
  two K=64 halves so each half of head A pairs with the opposite half of
  head B (again different row groups + banks -> concurrent).
  LN rsqrt = exp(-0.5*ln(var+eps)) so one ACT table set serves the kernel.
  attn@V for head A is interleaved jt-by-jt with the score pipeline; head B
  runs from its kept aT tiles afterward. Softmax 1/Z via ln -> K=1 ones
  matmul replication -> exp(-x) on ScalarE.
  Projection: per-ct chains over heads with both query chunks sharing each
  weight load; result staged through a work tile and DMA'd out per chunk.
"""

import sys

for _p in ("/opt/trn_rl_repo",):
    if _p not in sys.path:
        sys.path.insert(0, _p)

from contextlib import ExitStack

import numpy as np
import ml_dtypes

import concourse.bass as bass
import concourse.mybir as mybir
import concourse.tile as tile
from concourse.bass_utils import run_bass_kernel_spmd

F32 = mybir.dt.float32
BF16 = mybir.dt.bfloat16
F16 = mybir.dt.float16
AF = mybir.ActivationFunctionType
OP = mybir.AluOpType

B = 8
CH = 512
H = W = 32
NT = H * W          # 1024 tokens
HEADS = 8
HD = 64
EPS = 1e-6
P = 128
CT = CH // P        # 4 channel tiles
TT = NT // P        # 8 token tiles
IC = NT // 512      # 2 free-dim chunks of 512
STRIP_W = 60 * 32   # 1920
VW = 66             # per-head v stride: [v(64) | 1 | pad]


def _build_strips(rel: np.ndarray) -> np.ndarray:
    """(3969, 8) rel table -> (8, 128, 1920) bias strips.

    strip[h, 32*jh_l + jw, 32*g + iw] = T_h[g - jh_l + 3, iw - jw + 31]
    where T_h = rel[:, h].reshape(63, 63).
    bias.T block for key-tile jt is then strip[:, (28-4*jt)*32 : +1024].
    """
    T = rel.reshape(63, 63, HEADS)  # [a, b, h]
    jh_l = np.arange(4)[:, None, None, None]
    jw = np.arange(32)[None, :, None, None]
    g = np.arange(60)[None, None, :, None]
    iw = np.arange(32)[None, None, None, :]
    a = g - jh_l + 3          # in [0,62]
    b = iw - jw + 31          # in [0,62]
    a_b, b_b = np.broadcast_arrays(a, b)
    out = T[a_b, b_b, :]      # (4, 32, 60, 32, 8)
    out = np.ascontiguousarray(np.moveaxis(out, -1, 0)).reshape(HEADS, 128, STRIP_W)
    return out


def _build_nc() -> bass.Bass:
    nc = bass.Bass()

    x_d = nc.declare_dram_parameter("x", [CH, NT], F32, isOutput=False)
    xb_d = nc.declare_dram_parameter("xb", [CH, NT], BF16, isOutput=False)
    wqT_d = nc.declare_dram_parameter("wqT", [CH, CH], BF16, isOutput=False)
    wkT_d = nc.declare_dram_parameter("wkT", [CH, CH], BF16, isOutput=False)
    wvT_d = nc.declare_dram_parameter("wvT", [CH, CH], BF16, isOutput=False)
    wpP_d = nc.declare_dram_parameter("wpP", [HD, HEADS, CH], BF16, isOutput=False)
    bqk_d = nc.declare_dram_parameter("bqk", [2, CH], F32, isOutput=False)
    brow_d = nc.declare_dram_parameter("brow", [2, CH], BF16, isOutput=False)
    strips_d = nc.declare_dram_parameter("strips", [HEADS, P, STRIP_W], BF16,
                                         isOutput=False)
    y_d = nc.declare_dram_parameter("y", [CH, NT], F32, isOutput=True)

    with tile.TileContext(nc) as tc, ExitStack() as ctx:
        singles = ctx.enter_context(tc.tile_pool(name="singles", bufs=1))
        work = ctx.enter_context(tc.tile_pool(name="work", bufs=4))
        es_pool = ctx.enter_context(tc.tile_pool(name="es_pool", bufs=3))
        strip_pool = ctx.enter_context(tc.tile_pool(name="strip_pool", bufs=4))
        at_pool = ctx.enter_context(tc.tile_pool(name="at_pool", bufs=8))
        # PSUM budget (8 banks): psA (128,1024)x2bufs = 4 banks (LN stats +
        # scores); psB (128,512)x2 = 2 banks (qkv/proj/zrep); ps_o 2 banks.
        psA = ctx.enter_context(tc.tile_pool(name="psA", bufs=2, space="PSUM"))
        psB = ctx.enter_context(tc.tile_pool(name="psB", bufs=2, space="PSUM"))
        ps_o = ctx.enter_context(tc.tile_pool(name="ps_o", bufs=1, space="PSUM"))

        # ---------- persistent SBUF ----------
        x_sb = singles.tile([P, CT, NT], F32)        # residual source
        xb_sb = singles.tile([P, CT, NT], BF16)      # bf16 x for stats
        xn_sb = singles.tile([P, CT, NT], BF16)      # LN output
        qT_sb = singles.tile([P, CT, NT], BF16)      # (d part, t free)
        kT_sb = singles.tile([P, CT, NT], BF16)
        v_sb = singles.tile([P, TT, HEADS * VW], BF16)
        oTn_sb = singles.tile([HD, HEADS, NT], BF16)  # normalized per-head oT

        wq_sb = singles.tile([P, CT, CH], BF16)
        wk_sb = singles.tile([P, CT, CH], BF16)
        wv_sb = singles.tile([P, CT, CH], BF16)
        wpP_sb = singles.tile([HD, HEADS, CH], BF16)
        bqk_sb = singles.tile([P, 2, CT], F32)       # per-partition bias for q,k
        brow_sb = singles.tile([1, 2, CH], BF16)     # bv_eff, bp rows
        ones_mb = singles.tile([P, P], BF16)         # bf16 ones (LN stats lhsT)
        ones_rb = singles.tile([1, 512], BF16)       # bf16 ones row
        ones16 = singles.tile([HD + 1, HD], F16)     # f16 ones (zrep lhsT, row 64)
        lnz_sb = singles.tile([HD + 1, NT], F16)     # ln(Z) row at partition 64

        mu_b = singles.tile([P, NT], BF16)
        rs_b = singles.tile([P, NT], BF16)
        m2_f = singles.tile([P, NT], F32)
        ve_f = singles.tile([P, NT], F32)

        nc.vector.memset(ones_mb[:], 1.0)
        nc.vector.memset(ones_rb[:], 1.0)
        nc.vector.memset(ones16[:], 1.0)
        nc.sync.dma_start(bqk_sb[:], bqk_d.rearrange("i (o p) -> p i o", p=P))
        nc.sync.dma_start(brow_sb[:], brow_d[None, :, :])
        # per-ct x chunks so LN stats can start on the first chunk
        xb_r = xb_d.rearrange("(ct p) t -> p ct t", p=P)
        for ct in range(CT):
            nc.sync.dma_start(xb_sb[:, ct], xb_r[:, ct])
        nc.sync.dma_start(wq_sb[:], wqT_d.rearrange("(ck p) d -> p ck d", p=P))
        nc.sync.dma_start(wk_sb[:], wkT_d.rearrange("(ck p) d -> p ck d", p=P))
        nc.sync.dma_start(wv_sb[:], wvT_d.rearrange("(ck p) d -> p ck d", p=P))
        nc.sync.dma_start(wpP_sb[:], wpP_d[:])

        # ones columns of v
        v_view = v_sb[:].rearrange("p tt (h w) -> p tt h w", w=VW)
        nc.vector.memset(v_view[:, :, :, HD : HD + 1], 1.0)

        # ---------- phase 1: LayerNorm ----------
        with tc.tile_pool(name="ln_pool", bufs=2) as lnp:
            sum_ps = psA.tile([P, NT], F32, tag="big")
            sq_ps = psA.tile([P, NT], F32, tag="big")
            for ct in range(CT):
                x2 = lnp.tile([P, NT], BF16, name=f"x2_{ct}", tag="x2")
                nc.vector.tensor_tensor(out=x2[:], in0=xb_sb[:, ct],
                                        in1=xb_sb[:, ct], op=OP.mult)
                for ic in range(IC):
                    sl = slice(ic * 512, ic * 512 + 512)
                    nc.tensor.matmul(sum_ps[:, sl], lhsT=ones_mb[:],
                                     rhs=xb_sb[:, ct, sl],
                                     start=(ct == 0), stop=(ct == CT - 1))
                    nc.tensor.matmul(sq_ps[:, sl], lhsT=ones_mb[:], rhs=x2[:, sl],
                                     start=(ct == 0), stop=(ct == CT - 1))

            # mu (bf16 for the apply; bf16 is fine inside 512*mu^2 too)
            nc.scalar.activation(out=mu_b[:], in_=sum_ps[:], func=AF.Copy,
                                 scale=1.0 / CH)
            # 512*mu^2 ; (var+eps)*512 = (sq + 512*eps) - 512*mu^2
            nc.vector.tensor_tensor(out=m2_f[:], in0=mu_b[:], in1=sum_ps[:],
                                    op=OP.mult)
            nc.vector.scalar_tensor_tensor(out=ve_f[:], in0=sq_ps[:],
                                           scalar=float(CH * EPS), in1=m2_f[:],
                                           op0=OP.add, op1=OP.subtract)
            # rs = rsqrt(var+eps) = exp(-0.5*ln(var+eps)); keeps ACT on the
            # natural_log_exp table set for the entire kernel
            nc.scalar.activation(out=ve_f[:], in_=ve_f[:], func=AF.Ln,
                                 scale=1.0 / CH)
            nc.scalar.activation(out=rs_b[:], in_=ve_f[:], func=AF.Exp,
                                 scale=-0.5)

            for ct in range(CT):
                nc.vector.tensor_tensor(out=xn_sb[:, ct], in0=xb_sb[:, ct],
                                        in1=mu_b[:], op=OP.subtract)
                nc.vector.tensor_tensor(out=xn_sb[:, ct], in0=xn_sb[:, ct],
                                        in1=rs_b[:], op=OP.mult)

        # prefetch strips for the first head pair; residual x late (proj-only)
        strip_tiles = {}
        for h in (0, 1):
            st = strip_pool.tile([P, STRIP_W], BF16, name=f"strip{h}", tag="strip")
            nc.sync.dma_start(st[:], strips_d[h])
            strip_tiles[h] = st
        x_r = x_d.rearrange("(ct p) t -> p ct t", p=P)
        for ct in range(CT):
            nc.sync.dma_start(x_sb[:, ct], x_r[:, ct])

        # ---------- phase 2: Q, K, V projections ----------
        for dt in range(CT):
            dsl = slice(dt * P, dt * P + P)
            for ic in range(IC):
                sl = slice(ic * 512, ic * 512 + 512)
                q_ps = psB.tile([P, 512], F32, tag="small")
                for ck in range(CT):
                    nc.tensor.matmul(q_ps[:], lhsT=wq_sb[:, ck, dsl],
                                     rhs=xn_sb[:, ck, sl],
                                     start=(ck == 0), stop=(ck == CT - 1))
                nc.vector.tensor_scalar_add(out=qT_sb[:, dt, sl], in0=q_ps[:],
                                            scalar1=bqk_sb[:, 0, dt : dt + 1])
                k_ps = psB.tile([P, 512], F32, tag="small")
                for ck in range(CT):
                    nc.tensor.matmul(k_ps[:], lhsT=wk_sb[:, ck, dsl],
                                     rhs=xn_sb[:, ck, sl],
                                     start=(ck == 0), stop=(ck == CT - 1))
                nc.vector.tensor_scalar_add(out=kT_sb[:, dt, sl], in0=k_ps[:],
                                            scalar1=bqk_sb[:, 1, dt : dt + 1])

        for tt in range(TT):
            tsl = slice(tt * P, tt * P + P)
            v_ps = psB.tile([P, 512], F32, tag="small")
            for ck in range(CT):
                nc.tensor.matmul(v_ps[:], lhsT=xn_sb[:, ck, tsl],
                                 rhs=wv_sb[:, ck, :],
                                 start=(ck == 0), stop=False)
            nc.tensor.matmul(v_ps[:], lhsT=ones_rb[:, :P], rhs=brow_sb[:, 0, :],
                             start=False, stop=True)
            nc.vector.tensor_copy(
                out=v_view[:, tt, :, 0:HD],
                in_=v_ps[:].rearrange("p (h w) -> p h w", w=HD))

        # ---------- phase 3: attention, head pairs (A rows 0-63, B rows 64-127) --
        def zpath(h, o_ps):
            """replicate ln(Z) via K=1 matmul -> exp(-x) -> oTn."""
            for ic in range(IC):
                sl = slice(ic * 512, ic * 512 + 512)
                zl_ps = psB.tile([P, 512], F32, tag="small")
                nc.tensor.matmul(zl_ps[:HD, :], lhsT=ones16[HD : HD + 1, :],
                                 rhs=lnz_sb[HD : HD + 1, sl],
                                 start=True, stop=True)
                zrep = work.tile([HD, 512], F32, tag="zrep")
                nc.scalar.activation(out=zrep[:], in_=zl_ps[:HD, :], func=AF.Exp,
                                     scale=-1.0)
                nc.vector.tensor_tensor(out=oTn_sb[:, h, sl], in0=o_ps[:HD, sl],
                                        in1=zrep[:], op=OP.mult)

        # y_acc: residual + proj(heads 0-3), built mid-attention (pass 1)
        y_acc = singles.tile([P, CT, NT], F32)

        def proj_pass1():
            for ct in range(CT):
                csl = slice(ct * P, ct * P + P)
                for icc in range(IC):
                    sl = slice(icc * 512, icc * 512 + 512)
                    yp = psB.tile([P, 512], F32, tag="small",
                                  name=f"yp1_{ct}_{icc}")
                    for hh in range(4):
                        nc.tensor.matmul(yp[:], lhsT=wpP_sb[:, hh, csl],
                                         rhs=oTn_sb[:, hh, sl],
                                         start=(hh == 0), stop=False)
                    nc.tensor.matmul(yp[:], lhsT=brow_sb[:, 1, csl],
                                     rhs=ones_rb[:, :512],
                                     start=False, stop=True)
                    nc.vector.tensor_tensor(out=y_acc[:, ct, sl], in0=yp[:],
                                            in1=x_sb[:, ct, sl], op=OP.add)

        pending_z = None       # (head, o_ps) whose PE z-work is deferred
        for h in range(HEADS):
            dtl = h // 2
            drow = HD * (h % 2)
            strip = strip_tiles.pop(h)
            if h < HEADS - 2:
                st = strip_pool.tile([P, STRIP_W], BF16, name=f"strip{h + 2}",
                                     tag="strip")
                nc.sync.dma_start(st[:], strips_d[h + 2])
                strip_tiles[h + 2] = st

            at_tiles = []
            o_ps = None
            for jt in range(TT):
                s_ps = psA.tile([P, NT], F32, tag="big")
                off = (28 - 4 * jt) * 32
                for ic in range(IC):
                    sl = slice(ic * 512, ic * 512 + 512)
                    nc.tensor.matmul(
                        s_ps[:, sl],
                        lhsT=kT_sb[drow : drow + HD, dtl, jt * P : jt * P + P],
                        rhs=qT_sb[drow : drow + HD, dtl, sl],
                        start=True, stop=True)
                # exp(s) on ScalarE, then * exp(bias) on DVE (bf16 2x mode):
                # exp(s + b) = exp(s) * exp(b), strips hold exp(b) host-side
                eS = es_pool.tile([P, NT], BF16, name=f"eS_{h}_{jt}", tag="eS")
                nc.scalar.activation(out=eS[:], in_=s_ps[:], func=AF.Exp)
                aT = at_pool.tile([P, NT], BF16, name=f"aT_{h}_{jt}", tag="aT")
                nc.vector.tensor_tensor(out=aT[:], in0=eS[:],
                                        in1=strip[:, off : off + NT], op=OP.mult)
                at_tiles.append(aT)
                if jt == 0 and pending_z is not None:
                    # previous head's z-normalization matmuls, placed after
                    # this head's first score block so ln(Z) has drained
                    zpath(*pending_z)
                    pending_z = None
                if jt == 0 and h == 5:
                    proj_pass1()
                if jt > 0:
                    if jt == 1:
                        o_ps = ps_o.tile([HD + 1, NT], F32, tag="o",
                                         name=f"o_ps_{h}")
                    for ic in range(IC):
                        sl = slice(ic * 512, ic * 512 + 512)
                        nc.tensor.matmul(
                            o_ps[:, sl],
                            lhsT=v_sb[:, jt - 1, h * VW : h * VW + HD + 1],
                            rhs=at_tiles[jt - 1][:, sl],
                            start=(jt - 1 == 0), stop=False)
            for ic in range(IC):
                sl = slice(ic * 512, ic * 512 + 512)
                nc.tensor.matmul(o_ps[:, sl],
                                 lhsT=v_sb[:, TT - 1, h * VW : h * VW + HD + 1],
                                 rhs=at_tiles[TT - 1][:, sl],
                                 start=False, stop=True)
            nc.scalar.activation(out=lnz_sb[HD : HD + 1, :],
                                 in_=o_ps[HD : HD + 1, :], func=AF.Ln)
            pending_z = (h, o_ps)
        zpath(*pending_z)

        # ---------- phase 4: projection pass 2 (heads 4-7) + y_acc ----------
        for ct in range(CT):
            csl = slice(ct * P, ct * P + P)
            y_ps = [psB.tile([P, 512], F32, tag="small", name=f"y_ps_{ct}_{i}")
                    for i in range(IC)]
            for h in range(4, HEADS):
                for icc in range(IC):
                    sl = slice(icc * 512, icc * 512 + 512)
                    nc.tensor.matmul(y_ps[icc][:], lhsT=wpP_sb[:, h, csl],
                                     rhs=oTn_sb[:, h, sl],
                                     start=(h == 4), stop=(h == HEADS - 1))
            for icc in range(IC):
                sl = slice(icc * 512, icc * 512 + 512)
                yw = work.tile([P, 512], F32, tag="yw")
                nc.vector.tensor_tensor(out=yw[:], in0=y_ps[icc][:],
                                        in1=y_acc[:, ct, sl], op=OP.add)
                nc.sync.dma_start(y_d[csl, sl], yw[:])

    return nc


def _legalize_waits(nc, max_waits: int = 1):
    """Split multi-wait instructions into preceding same-engine NoOps.

    The TPB instruction encoding carries a single sync-wait slot and this
    walrus build refuses to legalize ("Too many sync wait commands"), so do
    it here: engines execute their queue in order, so a NoOp carrying one of
    the waits delays everything after it on that engine identically.
    """
    import orjson

    data = orjson.loads(mybir.module_to_json_bytes(nc.m))
    ctr = [0]

    def fix_block(block):
        out = []
        for inst in block.get("instructions", []):
            si = inst.get("sync_info") or {}
            waits = si.get("on_wait") or []
            if len(waits) > max_waits:
                for w in waits[max_waits:]:
                    ctr[0] += 1
                    nop = {
                        "name": f"I-WS{ctr[0]}",
                        "opcode": "NoOp",
                        "engine": inst["engine"],
                        "ins": [],
                        "outs": [],
                        "sync_info": {"on_wait": [w], "on_update": []},
                    }
                    if "debug" in inst:
                        nop["debug"] = inst["debug"]
                    out.append(nop)
                si = dict(si)
                si["on_wait"] = waits[:max_waits]
                inst["sync_info"] = si
            out.append(inst)
        block["instructions"] = out
        for b in block.get("blocks", []):
            fix_block(b)

    for fn in data["functions"]:
        for b in fn.get("blocks", []):
            fix_block(b)
    nc.m = mybir.module_from_json_bytes(orjson.dumps(data))
    return nc


_NC = None

BF = ml_dtypes.bfloat16


def _host_prep(x, norm_w, norm_b, wq, bq, wk, bk, wv, bv, wp, bp, rel):
    scale = HD ** -0.5
    # fold LN affine + score scale into the projection weights (exact algebra)
    wq_eff = (wq * norm_w[None, :]) * scale
    bq_eff = (bq + wq @ norm_b) * scale
    wk_eff = wk * norm_w[None, :]
    bk_eff = bk + wk @ norm_b
    wv_eff = wv * norm_w[None, :]
    bv_eff = bv + wv @ norm_b

    wqT = np.ascontiguousarray(wq_eff.T).astype(BF)
    wkT = np.ascontiguousarray(wk_eff.T).astype(BF)
    wvT = np.ascontiguousarray(wv_eff.T).astype(BF)
    # wp permuted so each head's 64 input rows sit at partitions 0..63
    wpP = np.ascontiguousarray(
        wp.T.reshape(HEADS, HD, CH).transpose(1, 0, 2)).astype(BF)

    bqk = np.stack([bq_eff, bk_eff]).astype(np.float32)
    brow = np.stack([bv_eff, bp]).astype(BF)
    strips = np.exp(_build_strips(np.asarray(rel, np.float32))).astype(BF)

    shared = {
        "wqT": wqT, "wkT": wkT, "wvT": wvT, "wpP": wpP,
        "bqk": bqk, "brow": brow, "strips": strips,
    }
    in_maps = []
    for b in range(B):
        m = dict(shared)
        xf = np.ascontiguousarray(x[b].reshape(CH, NT)).astype(np.float32)
        m["x"] = xf
        m["xb"] = xf.astype(BF)
        in_maps.append(m)
    return in_maps


def kernel(**inputs):
    global _NC
    if _NC is None:
        _NC = _legalize_waits(_build_nc())
    in_maps = _host_prep(**{k: np.asarray(v) for k, v in inputs.items()})
    res = run_bass_kernel_spmd(_NC, in_maps, list(range(B)))
    out = np.stack([res.results[b]["y"].reshape(CH, H, W) for b in range(B)])
    return out.astype(np.float32)


if __name__ == "__main__":
    nc = _build_nc()
    print("built OK")


# revision 12
# speedup vs baseline: 1.1698x; 1.0217x over previous
"""Trainium2 Bass kernel for nn_MHAAttention (LayerNorm2d + MHA w/ rel-pos bias + residual).

Sharding: data-parallel over batch - 8 batch elements, one per NeuronCore.
No collectives needed.

v3 (bf16 + row-packed head pairs):
  all matmuls bf16 (single-pass PE). Heads processed in pairs (2p, 2p+1):
  head A lives at array rows 0-63, head B at rows 64-127, so their K=64
  score matmuls execute CONCURRENTLY (different row groups + PSUM banks).
  The rel-pos bias is accumulated into PSUM by identity matmuls, split into
  two K=64 halves so each half of head A pairs with the opposite half of
  head B (again different row groups + banks -> concurrent).
  LN rsqrt = exp(-0.5*ln(var+eps)) so one ACT table set serves the kernel.
  attn@V for head A is interleaved jt-by-jt with the score pipeline; head B
  runs from its kept aT tiles afterward. Softmax 1/Z via ln -> K=1 ones
  matmul replication -> exp(-x) on ScalarE.
  Projection: per-ct chains over heads with both query chunks sharing each
  weight load; result staged through a work tile and DMA'd out per chunk.
"""

import sys

for _p in ("/opt/trn_rl_repo",):
    if _p not in sys.path:
        sys.path.insert(0, _p)

from contextlib import ExitStack

import numpy as np
import ml_dtypes

import concourse.bass as bass
import concourse.mybir as mybir
import concourse.tile as tile
from concourse.bass_utils import run_bass_kernel_spmd

F32 = mybir.dt.float32
BF16 = mybir.dt.bfloat16
F16 = mybir.dt.float16
AF = mybir.ActivationFunctionType
OP = mybir.AluOpType

B = 8
CH = 512
H = W = 32
NT = H * W          # 1024 tokens
HEADS = 8
HD = 64
EPS = 1e-6
P = 128
CT = CH // P        # 4 channel tiles
TT = NT // P        # 8 token tiles
IC = NT // 512      # 2 free-dim chunks of 512
STRIP_W = 60 * 32   # 1920
VW = 66             # per-head v stride: [v(64) | 1 | pad]


def _build_strips(rel: np.ndarray) -> np.ndarray:
    """(3969, 8) rel table -> (8, 128, 1920) bias strips.

    strip[h, 32*jh_l + jw, 32*g + iw] = T_h[g - jh_l + 3, iw - jw + 31]
    where T_h = rel[:, h].reshape(63, 63).
    bias.T block for key-tile jt is then strip[:, (28-4*jt)*32 : +1024].
    """
    T = rel.reshape(63, 63, HEADS)  # [a, b, h]
    jh_l = np.arange(4)[:, None, None, None]
    jw = np.arange(32)[None, :, None, None]
    g = np.arange(60)[None, None, :, None]
    iw = np.arange(32)[None, None, None, :]
    a = g - jh_l + 3          # in [0,62]
    b = iw - jw + 31          # in [0,62]
    a_b, b_b = np.broadcast_arrays(a, b)
    out = T[a_b, b_b, :]      # (4, 32, 60, 32, 8)
    out = np.ascontiguousarray(np.moveaxis(out, -1, 0)).reshape(HEADS, 128, STRIP_W)
    return out


def _build_nc() -> bass.Bass:
    nc = bass.Bass()

    x_d = nc.declare_dram_parameter("x", [CH, NT], F32, isOutput=False)
    xb_d = nc.declare_dram_parameter("xb", [CH, NT], BF16, isOutput=False)
    wqT_d = nc.declare_dram_parameter("wqT", [CH, CH], BF16, isOutput=False)
    wkT_d = nc.declare_dram_parameter("wkT", [CH, CH], BF16, isOutput=False)
    wvT_d = nc.declare_dram_parameter("wvT", [CH, CH], BF16, isOutput=False)
    wpP_d = nc.declare_dram_parameter("wpP", [HD, HEADS, CH], BF16, isOutput=False)
    bqk_d = nc.declare_dram_parameter("bqk", [2, CH], F32, isOutput=False)
    brow_d = nc.declare_dram_parameter("brow", [2, CH], BF16, isOutput=False)
    strips_d = nc.declare_dram_parameter("strips", [HEADS, P, STRIP_W], BF16,
                                         isOutput=False)
    y_d = nc.declare_dram_parameter("y", [CH, NT], F32, isOutput=True)

    with tile.TileContext(nc) as tc, ExitStack() as ctx:
        singles = ctx.enter_context(tc.tile_pool(name="singles", bufs=1))
        work = ctx.enter_context(tc.tile_pool(name="work", bufs=4))
        es_pool = ctx.enter_context(tc.tile_pool(name="es_pool", bufs=3))
        strip_pool = ctx.enter_context(tc.tile_pool(name="strip_pool", bufs=4))
        at_pool = ctx.enter_context(tc.tile_pool(name="at_pool", bufs=8))
        # PSUM budget (8 banks): psA (128,1024)x2bufs = 4 banks (LN stats +
        # scores); psB (128,512)x2 = 2 banks (qkv/proj/zrep); ps_o 2 banks.
        psA = ctx.enter_context(tc.tile_pool(name="psA", bufs=2, space="PSUM"))
        psB = ctx.enter_context(tc.tile_pool(name="psB", bufs=2, space="PSUM"))
        ps_o = ctx.enter_context(tc.tile_pool(name="ps_o", bufs=1, space="PSUM"))

        # ---------- persistent SBUF ----------
        x_sb = singles.tile([P, CT, NT], F32)        # residual source
        xb_sb = singles.tile([P, CT, NT], BF16)      # bf16 x for stats
        xn_sb = singles.tile([P, CT, NT], BF16)      # LN output
        qT_sb = singles.tile([P, CT, NT], BF16)      # (d part, t free)
        kT_sb = singles.tile([P, CT, NT], BF16)
        v_sb = singles.tile([P, TT, HEADS * VW], BF16)
        oTn_sb = singles.tile([HD, HEADS, NT], BF16)  # normalized per-head oT

        wq_sb = singles.tile([P, CT, CH], BF16)
        wk_sb = singles.tile([P, CT, CH], BF16)
        wv_sb = singles.tile([P, CT, CH], BF16)
        wpP_sb = singles.tile([HD, HEADS, CH], BF16)
        bqk_sb = singles.tile([P, 2, CT], F32)       # per-partition bias for q,k
        brow_sb = singles.tile([1, 2, CH], BF16)     # bv_eff, bp rows
        ones_mb = singles.tile([P, P], BF16)         # bf16 ones (LN stats lhsT)
        ones_rb = singles.tile([1, 512], BF16)       # bf16 ones row
        ones16 = singles.tile([HD + 1, HD], F16)     # f16 ones (zrep lhsT, row 64)
        lnz_sb = singles.tile([HD + 1, NT], F16)     # ln(Z) row at partition 64

        mu_b = singles.tile([P, NT], BF16)
        rs_b = singles.tile([P, NT], BF16)
        m2_f = singles.tile([P, NT], F32)
        ve_f = singles.tile([P, NT], F32)

        nc.vector.memset(ones_mb[:], 1.0)
        nc.vector.memset(ones_rb[:], 1.0)
        nc.vector.memset(ones16[:], 1.0)
        nc.sync.dma_start(bqk_sb[:], bqk_d.rearrange("i (o p) -> p i o", p=P))
        nc.sync.dma_start(brow_sb[:], brow_d[None, :, :])
        # per-ct x chunks so LN stats can start on the first chunk
        xb_r = xb_d.rearrange("(ct p) t -> p ct t", p=P)
        for ct in range(CT):
            nc.sync.dma_start(xb_sb[:, ct], xb_r[:, ct])
        nc.sync.dma_start(wq_sb[:], wqT_d.rearrange("(ck p) d -> p ck d", p=P))
        nc.sync.dma_start(wk_sb[:], wkT_d.rearrange("(ck p) d -> p ck d", p=P))
        nc.sync.dma_start(wv_sb[:], wvT_d.rearrange("(ck p) d -> p ck d", p=P))
        nc.sync.dma_start(wpP_sb[:], wpP_d[:])

        # ones columns of v
        v_view = v_sb[:].rearrange("p tt (h w) -> p tt h w", w=VW)
        nc.vector.memset(v_view[:, :, :, HD : HD + 1], 1.0)

        # ---------- phase 1: LayerNorm ----------
        with tc.tile_pool(name="ln_pool", bufs=2) as lnp:
            sum_ps = psA.tile([P, NT], F32, tag="big")
            sq_ps = psA.tile([P, NT], F32, tag="big")
            for ct in range(CT):
                x2 = lnp.tile([P, NT], BF16, name=f"x2_{ct}", tag="x2")
                nc.vector.tensor_tensor(out=x2[:], in0=xb_sb[:, ct],
                                        in1=xb_sb[:, ct], op=OP.mult)
                for ic in range(IC):
                    sl = slice(ic * 512, ic * 512 + 512)
                    nc.tensor.matmul(sum_ps[:, sl], lhsT=ones_mb[:],
                                     rhs=xb_sb[:, ct, sl],
                                     start=(ct == 0), stop=(ct == CT - 1))
                    nc.tensor.matmul(sq_ps[:, sl], lhsT=ones_mb[:], rhs=x2[:, sl],
                                     start=(ct == 0), stop=(ct == CT - 1))

            # mu (bf16 for the apply; bf16 is fine inside 512*mu^2 too)
            nc.scalar.activation(out=mu_b[:], in_=sum_ps[:], func=AF.Copy,
                                 scale=1.0 / CH)
            # 512*mu^2 ; (var+eps)*512 = (sq + 512*eps) - 512*mu^2
            nc.vector.tensor_tensor(out=m2_f[:], in0=mu_b[:], in1=sum_ps[:],
                                    op=OP.mult)
            nc.vector.scalar_tensor_tensor(out=ve_f[:], in0=sq_ps[:],
                                           scalar=float(CH * EPS), in1=m2_f[:],
                                           op0=OP.add, op1=OP.subtract)
            # rs = rsqrt(var+eps) = exp(-0.5*ln(var+eps)); keeps ACT on the
            # natural_log_exp table set for the entire kernel
            nc.scalar.activation(out=ve_f[:], in_=ve_f[:], func=AF.Ln,
                                 scale=1.0 / CH)
            nc.scalar.activation(out=rs_b[:], in_=ve_f[:], func=AF.Exp,
                                 scale=-0.5)

            for ct in range(CT):
                nc.vector.tensor_tensor(out=xn_sb[:, ct], in0=xb_sb[:, ct],
                                        in1=mu_b[:], op=OP.subtract)
                nc.vector.tensor_tensor(out=xn_sb[:, ct], in0=xn_sb[:, ct],
                                        in1=rs_b[:], op=OP.mult)

        # prefetch strips for the first head pair; residual x late (proj-only)
        strip_tiles = {}
        for h in (0, 1):
            st = strip_pool.tile([P, STRIP_W], BF16, name=f"strip{h}", tag="strip")
            nc.sync.dma_start(st[:], strips_d[h])
            strip_tiles[h] = st
        x_r = x_d.rearrange("(ct p) t -> p ct t", p=P)
        for ct in range(CT):
            nc.sync.dma_start(x_sb[:, ct], x_r[:, ct])

        # ---------- phase 2: Q, K, V projections ----------
        for dt in range(CT):
            dsl = slice(dt * P, dt * P + P)
            for ic in range(IC):
                sl = slice(ic * 512, ic * 512 + 512)
                q_ps = psB.tile([P, 512], F32, tag="small")
                for ck in range(CT):
                    nc.tensor.matmul(q_ps[:], lhsT=wq_sb[:, ck, dsl],
                                     rhs=xn_sb[:, ck, sl],
                                     start=(ck == 0), stop=(ck == CT - 1))
                nc.vector.tensor_scalar_add(out=qT_sb[:, dt, sl], in0=q_ps[:],
                                            scalar1=bqk_sb[:, 0, dt : dt + 1])
                k_ps = psB.tile([P, 512], F32, tag="small")
                for ck in range(CT):
                    nc.tensor.matmul(k_ps[:], lhsT=wk_sb[:, ck, dsl],
                                     rhs=xn_sb[:, ck, sl],
                                     start=(ck == 0), stop=(ck == CT - 1))
                nc.vector.tensor_scalar_add(out=kT_sb[:, dt, sl], in0=k_ps[:],
                                            scalar1=bqk_sb[:, 1, dt : dt + 1])

        for tt in range(TT):
            tsl = slice(tt * P, tt * P + P)
            v_ps = psB.tile([P, 512], F32, tag="small")
            for ck in range(CT):
                nc.tensor.matmul(v_ps[:], lhsT=xn_sb[:, ck, tsl],
                                 rhs=wv_sb[:, ck, :],
                                 start=(ck == 0), stop=False)
            nc.tensor.matmul(v_ps[:], lhsT=ones_rb[:, :P], rhs=brow_sb[:, 0, :],
                             start=False, stop=True)
            nc.vector.tensor_copy(
                out=v_view[:, tt, :, 0:HD],
                in_=v_ps[:].rearrange("p (h w) -> p h w", w=HD))

        # ---------- phase 3: attention, head pairs (A rows 0-63, B rows 64-127) --
        def zpath(h, o_ps):
            """replicate ln(Z) via K=1 matmul -> exp(-x) -> oTn."""
            for ic in range(IC):
                sl = slice(ic * 512, ic * 512 + 512)
                zl_ps = psB.tile([P, 512], F32, tag="small")
                nc.tensor.matmul(zl_ps[:HD, :], lhsT=ones16[HD : HD + 1, :],
                                 rhs=lnz_sb[HD : HD + 1, sl],
                                 start=True, stop=True)
                zrep = work.tile([HD, 512], F32, tag="zrep")
                nc.scalar.activation(out=zrep[:], in_=zl_ps[:HD, :], func=AF.Exp,
                                     scale=-1.0)
                nc.vector.tensor_tensor(out=oTn_sb[:, h, sl], in0=o_ps[:HD, sl],
                                        in1=zrep[:], op=OP.mult)

        # y_acc: residual + proj(heads 0-3), built mid-attention (pass 1)
        y_acc = singles.tile([P, CT, NT], F32)

        def proj_pass1(ct):
                csl = slice(ct * P, ct * P + P)
                for icc in range(IC):
                    sl = slice(icc * 512, icc * 512 + 512)
                    yp = psB.tile([P, 512], F32, tag="small",
                                  name=f"yp1_{ct}_{icc}")
                    for hh in range(4):
                        nc.tensor.matmul(yp[:], lhsT=wpP_sb[:, hh, csl],
                                         rhs=oTn_sb[:, hh, sl],
                                         start=(hh == 0), stop=False)
                    nc.tensor.matmul(yp[:], lhsT=brow_sb[:, 1, csl],
                                     rhs=ones_rb[:, :512],
                                     start=False, stop=True)
                    nc.vector.tensor_tensor(out=y_acc[:, ct, sl], in0=yp[:],
                                            in1=x_sb[:, ct, sl], op=OP.add)

        pending_z = None       # (head, o_ps) whose PE z-work is deferred
        for h in range(HEADS):
            dtl = h // 2
            drow = HD * (h % 2)
            strip = strip_tiles.pop(h)
            if h < HEADS - 2:
                st = strip_pool.tile([P, STRIP_W], BF16, name=f"strip{h + 2}",
                                     tag="strip")
                nc.sync.dma_start(st[:], strips_d[h + 2])
                strip_tiles[h + 2] = st

            at_tiles = []
            o_ps = None
            for jt in range(TT):
                s_ps = psA.tile([P, NT], F32, tag="big")
                off = (28 - 4 * jt) * 32
                for ic in range(IC):
                    sl = slice(ic * 512, ic * 512 + 512)
                    nc.tensor.matmul(
                        s_ps[:, sl],
                        lhsT=kT_sb[drow : drow + HD, dtl, jt * P : jt * P + P],
                        rhs=qT_sb[drow : drow + HD, dtl, sl],
                        start=True, stop=True)
                # exp(s) on ScalarE, then * exp(bias) on DVE (bf16 2x mode):
                # exp(s + b) = exp(s) * exp(b), strips hold exp(b) host-side
                eS = es_pool.tile([P, NT], BF16, name=f"eS_{h}_{jt}", tag="eS")
                nc.scalar.activation(out=eS[:], in_=s_ps[:], func=AF.Exp)
                aT = at_pool.tile([P, NT], BF16, name=f"aT_{h}_{jt}", tag="aT")
                nc.vector.tensor_tensor(out=aT[:], in0=eS[:],
                                        in1=strip[:, off : off + NT], op=OP.mult)
                at_tiles.append(aT)
                if jt == 0 and pending_z is not None:
                    # previous head's z-normalization matmuls, placed after
                    # this head's first score block so ln(Z) has drained
                    zpath(*pending_z)
                    pending_z = None
                if jt == 2 and h >= 4:
                    proj_pass1(h - 4)
                if jt > 1:
                    if jt == 2:
                        o_ps = ps_o.tile([HD + 1, NT], F32, tag="o",
                                         name=f"o_ps_{h}")
                    for ic in range(IC):
                        sl = slice(ic * 512, ic * 512 + 512)
                        nc.tensor.matmul(
                            o_ps[:, sl],
                            lhsT=v_sb[:, jt - 2, h * VW : h * VW + HD + 1],
                            rhs=at_tiles[jt - 2][:, sl],
                            start=(jt - 2 == 0), stop=False)
            for jl in (TT - 2, TT - 1):
                for ic in range(IC):
                    sl = slice(ic * 512, ic * 512 + 512)
                    nc.tensor.matmul(o_ps[:, sl],
                                     lhsT=v_sb[:, jl, h * VW : h * VW + HD + 1],
                                     rhs=at_tiles[jl][:, sl],
                                     start=False, stop=(jl == TT - 1))
            nc.scalar.activation(out=lnz_sb[HD : HD + 1, :],
                                 in_=o_ps[HD : HD + 1, :], func=AF.Ln)
            pending_z = (h, o_ps)
        zpath(*pending_z)

        # keep the PE array's HAM clock-gate warm across the z-path tail so
        # projection pass 2 runs at 2.4 GHz (results discarded)
        warm_ps = psA.tile([P, NT], F32, tag="big")
        for wi in range(8):
            nc.tensor.matmul(warm_ps[:, 0:512], lhsT=ones_mb[:],
                             rhs=xn_sb[:, 0, 0:512], start=True, stop=True)

        # ---------- phase 4: projection pass 2 (heads 4-7) + y_acc ----------
        for ct in range(CT):
            csl = slice(ct * P, ct * P + P)
            y_ps = [psB.tile([P, 512], F32, tag="small", name=f"y_ps_{ct}_{i}")
                    for i in range(IC)]
            for h in range(4, HEADS):
                for icc in range(IC):
                    sl = slice(icc * 512, icc * 512 + 512)
                    nc.tensor.matmul(y_ps[icc][:], lhsT=wpP_sb[:, h, csl],
                                     rhs=oTn_sb[:, h, sl],
                                     start=(h == 4), stop=(h == HEADS - 1))
            for icc in range(IC):
                sl = slice(icc * 512, icc * 512 + 512)
                yw = work.tile([P, 512], F32, tag="yw")
                nc.vector.tensor_tensor(out=yw[:], in0=y_ps[icc][:],
                                        in1=y_acc[:, ct, sl], op=OP.add)
                nc.sync.dma_start(y_d[csl, sl], yw[:])

    return nc


def _legalize_waits(nc, max_waits: int = 1):
    """Split multi-wait instructions into preceding same-engine NoOps.

    The TPB instruction encoding carries a single sync-wait slot and this
    walrus build refuses to legalize ("Too many sync wait commands"), so do
    it here: engines execute their queue in order, so a NoOp carrying one of
    the waits delays everything after it on that engine identically.
    """
    import orjson

    data = orjson.loads(mybir.module_to_json_bytes(nc.m))
    ctr = [0]

    def fix_block(block):
        out = []
        for inst in block.get("instructions", []):
            si = inst.get("sync_info") or {}
            waits = si.get("on_wait") or []
            if len(waits) > max_waits:
                for w in waits[max_waits:]:
                    ctr[0] += 1
                    nop = {
                        "name": f"I-WS{ctr[0]}",
                        "opcode": "NoOp",
                        "engine": inst["engine"],
                        "ins": [],
                        "outs": [],
                        "sync_info": {"on_wait": [w], "on_update": []},
                    }
                    if "debug" in inst:
                        nop["debug"] = inst["debug"]
                    out.append(nop)
                si = dict(si)
                si["on_wait"] = waits[:max_waits]
                inst["sync_info"] = si
            out.append(inst)
        block["instructions"] = out
        for b in block.get("blocks", []):
            fix_block(b)

    for fn in data["functions"]:
        for b in fn.get("blocks", []):
            fix_block(b)
    nc.m = mybir.module_from_json_bytes(orjson.dumps(data))
    return nc


_NC = None

BF = ml_dtypes.bfloat16


def _host_prep(x, norm_w, norm_b, wq, bq, wk, bk, wv, bv, wp, bp, rel):
    scale = HD ** -0.5
    # fold LN affine + score scale into the projection weights (exact algebra)
    wq_eff = (wq * norm_w[None, :]) * scale
    bq_eff = (bq + wq @ norm_b) * scale
    wk_eff = wk * norm_w[None, :]
    bk_eff = bk + wk @ norm_b
    wv_eff = wv * norm_w[None, :]
    bv_eff = bv + wv @ norm_b

    wqT = np.ascontiguousarray(wq_eff.T).astype(BF)
    wkT = np.ascontiguousarray(wk_eff.T).astype(BF)
    wvT = np.ascontiguousarray(wv_eff.T).astype(BF)
    # wp permuted so each head's 64 input rows sit at partitions 0..63
    wpP = np.ascontiguousarray(
        wp.T.reshape(HEADS, HD, CH).transpose(1, 0, 2)).astype(BF)

    bqk = np.stack([bq_eff, bk_eff]).astype(np.float32)
    brow = np.stack([bv_eff, bp]).astype(BF)
    strips = np.exp(_build_strips(np.asarray(rel, np.float32))).astype(BF)

    shared = {
        "wqT": wqT, "wkT": wkT, "wvT": wvT, "wpP": wpP,
        "bqk": bqk, "brow": brow, "strips": strips,
    }
    in_maps = []
    for b in range(B):
        m = dict(shared)
        xf = np.ascontiguousarray(x[b].reshape(CH, NT)).astype(np.float32)
        m["x"] = xf
        m["xb"] = xf.astype(BF)
        in_maps.append(m)
    return in_maps


def kernel(**inputs):
    global _NC
    if _NC is None:
        _NC = _legalize_waits(_build_nc())
    in_maps = _host_prep(**{k: np.asarray(v) for k, v in inputs.items()})
    res = run_bass_kernel_spmd(_NC, in_maps, list(range(B)))
    out = np.stack([res.results[b]["y"].reshape(CH, H, W) for b in range(B)])
    return out.astype(np.float32)


if __name__ == "__main__":
    nc = _build_nc()
    print("built OK")


# revision 14
# speedup vs baseline: 1.1846x; 1.0126x over previous
"""Trainium2 Bass kernel for nn_MHAAttention (LayerNorm2d + MHA w/ rel-pos bias + residual).

Sharding: data-parallel over batch - 8 batch elements, one per NeuronCore.
No collectives needed.

v3 (bf16 + row-packed head pairs):
  all matmuls bf16 (single-pass PE). Heads processed in pairs (2p, 2p+1):
  head A lives at array rows 0-63, head B at rows 64-127, so their K=64
  score matmuls execute CONCURRENTLY (different row groups + PSUM banks).
  The rel-pos bias is accumulated into PSUM by identity matmuls, split into
  two K=64 halves so each half of head A pairs with the opposite half of
  head B (again different row groups + banks -> concurrent).
  LN rsqrt = exp(-0.5*ln(var+eps)) so one ACT table set serves the kernel.
  attn@V for head A is interleaved jt-by-jt with the score pipeline; head B
  runs from its kept aT tiles afterward. Softmax 1/Z via ln -> K=1 ones
  matmul replication -> exp(-x) on ScalarE.
  Projection: per-ct chains over heads with both query chunks sharing each
  weight load; result staged through a work tile and DMA'd out per chunk.
"""

import sys

for _p in ("/opt/trn_rl_repo",):
    if _p not in sys.path:
        sys.path.insert(0, _p)

from contextlib import ExitStack

import numpy as np
import ml_dtypes

import concourse.bass as bass
import concourse.mybir as mybir
import concourse.tile as tile
from concourse.bass_utils import run_bass_kernel_spmd

F32 = mybir.dt.float32
BF16 = mybir.dt.bfloat16
F16 = mybir.dt.float16
AF = mybir.ActivationFunctionType
OP = mybir.AluOpType

B = 8
CH = 512
H = W = 32
NT = H * W          # 1024 tokens
HEADS = 8
HD = 64
EPS = 1e-6
P = 128
CT = CH // P        # 4 channel tiles
TT = NT // P        # 8 token tiles
IC = NT // 512      # 2 free-dim chunks of 512
STRIP_W = 60 * 32   # 1920
VW = 66             # per-head v stride: [v(64) | 1 | pad]


def _build_strips(rel: np.ndarray) -> np.ndarray:
    """(3969, 8) rel table -> (8, 128, 1920) bias strips.

    strip[h, 32*jh_l + jw, 32*g + iw] = T_h[g - jh_l + 3, iw - jw + 31]
    where T_h = rel[:, h].reshape(63, 63).
    bias.T block for key-tile jt is then strip[:, (28-4*jt)*32 : +1024].
    """
    T = rel.reshape(63, 63, HEADS)  # [a, b, h]
    jh_l = np.arange(4)[:, None, None, None]
    jw = np.arange(32)[None, :, None, None]
    g = np.arange(60)[None, None, :, None]
    iw = np.arange(32)[None, None, None, :]
    a = g - jh_l + 3          # in [0,62]
    b = iw - jw + 31          # in [0,62]
    a_b, b_b = np.broadcast_arrays(a, b)
    out = T[a_b, b_b, :]      # (4, 32, 60, 32, 8)
    out = np.ascontiguousarray(np.moveaxis(out, -1, 0)).reshape(HEADS, 128, STRIP_W)
    return out


def _build_nc() -> bass.Bass:
    nc = bass.Bass()

    x_d = nc.declare_dram_parameter("x", [CH, NT], F32, isOutput=False)
    xb_d = nc.declare_dram_parameter("xb", [CH, NT], BF16, isOutput=False)
    wqT_d = nc.declare_dram_parameter("wqT", [CH, CH], BF16, isOutput=False)
    wkT_d = nc.declare_dram_parameter("wkT", [CH, CH], BF16, isOutput=False)
    wvT_d = nc.declare_dram_parameter("wvT", [CH, CH], BF16, isOutput=False)
    wpP_d = nc.declare_dram_parameter("wpP", [HD, HEADS, CH], BF16, isOutput=False)
    bqk_d = nc.declare_dram_parameter("bqk", [2, CH], F32, isOutput=False)
    brow_d = nc.declare_dram_parameter("brow", [2, CH], BF16, isOutput=False)
    strips_d = nc.declare_dram_parameter("strips", [HEADS, P, STRIP_W], BF16,
                                         isOutput=False)
    y_d = nc.declare_dram_parameter("y", [CH, NT], F32, isOutput=True)

    with tile.TileContext(nc) as tc, ExitStack() as ctx:
        singles = ctx.enter_context(tc.tile_pool(name="singles", bufs=1))
        work = ctx.enter_context(tc.tile_pool(name="work", bufs=4))
        es_pool = ctx.enter_context(tc.tile_pool(name="es_pool", bufs=3))
        strip_pool = ctx.enter_context(tc.tile_pool(name="strip_pool", bufs=4))
        at_pool = ctx.enter_context(tc.tile_pool(name="at_pool", bufs=8))
        # PSUM budget (8 banks): psA (128,1024)x3bufs = 6 banks (everything
        # except the attn@V accumulator); ps_o 2 banks.
        psA = ctx.enter_context(tc.tile_pool(name="psA", bufs=2, space="PSUM"))
        psB = ctx.enter_context(tc.tile_pool(name="psB", bufs=2, space="PSUM"))
        ps_o = ctx.enter_context(tc.tile_pool(name="ps_o", bufs=1, space="PSUM"))

        # ---------- persistent SBUF ----------
        x_sb = singles.tile([P, CT, NT], F32)        # residual source
        xb_sb = singles.tile([P, CT, NT], BF16)      # bf16 x for stats
        xn_sb = singles.tile([P, CT, NT], BF16)      # LN output
        qT_sb = singles.tile([P, CT, NT], BF16)      # (d part, t free)
        kT_sb = singles.tile([P, CT, NT], BF16)
        v_sb = singles.tile([P, TT, HEADS * VW], BF16)
        oTn_sb = singles.tile([HD, HEADS, NT], BF16)  # normalized per-head oT

        wq_sb = singles.tile([P, CT, CH], BF16)
        wk_sb = singles.tile([P, CT, CH], BF16)
        wv_sb = singles.tile([P, CT, CH], BF16)
        wpP_sb = singles.tile([HD, HEADS, CH], BF16)
        bqk_sb = singles.tile([P, 2, CT], F32)       # per-partition bias for q,k
        brow_sb = singles.tile([1, 2, CH], BF16)     # bv_eff, bp rows
        ones_mb = singles.tile([P, P], BF16)         # bf16 ones (LN stats lhsT)
        ones_rb = singles.tile([1, 512], BF16)       # bf16 ones row
        ones16 = singles.tile([HD + 1, HD], F16)     # f16 ones (zrep lhsT, row 64)
        lnz_sb = singles.tile([HD + 1, NT], F16)     # ln(Z) row at partition 64

        mu_b = singles.tile([P, NT], BF16)
        rs_b = singles.tile([P, NT], BF16)
        m2_f = singles.tile([P, NT], F32)
        ve_f = singles.tile([P, NT], F32)

        nc.vector.memset(ones_mb[:], 1.0)
        nc.vector.memset(ones_rb[:], 1.0)
        nc.vector.memset(ones16[:], 1.0)
        # per-ct x chunks so LN stats can start on the first chunk;
        # q weights early so the QKV phase is not gated on DMA
        xb_r = xb_d.rearrange("(ct p) t -> p ct t", p=P)
        for ct in range(CT):
            nc.sync.dma_start(xb_sb[:, ct], xb_r[:, ct])
        nc.sync.dma_start(bqk_sb[:], bqk_d.rearrange("i (o p) -> p i o", p=P))
        nc.sync.dma_start(brow_sb[:], brow_d[None, :, :])
        nc.sync.dma_start(wq_sb[:], wqT_d.rearrange("(ck p) d -> p ck d", p=P))
        nc.sync.dma_start(wk_sb[:], wkT_d.rearrange("(ck p) d -> p ck d", p=P))
        nc.sync.dma_start(wv_sb[:], wvT_d.rearrange("(ck p) d -> p ck d", p=P))
        nc.sync.dma_start(wpP_sb[:], wpP_d[:])

        # ones columns of v
        v_view = v_sb[:].rearrange("p tt (h w) -> p tt h w", w=VW)
        nc.vector.memset(v_view[:, :, :, HD : HD + 1], 1.0)

        # ---------- phase 1: LayerNorm ----------
        with tc.tile_pool(name="ln_pool", bufs=2) as lnp:
            sum_ps = psA.tile([P, NT], F32, tag="big")
            sq_ps = psA.tile([P, NT], F32, tag="big")
            for ct in range(CT):
                x2 = lnp.tile([P, NT], BF16, name=f"x2_{ct}", tag="x2")
                nc.vector.tensor_tensor(out=x2[:], in0=xb_sb[:, ct],
                                        in1=xb_sb[:, ct], op=OP.mult)
                for ic in range(IC):
                    sl = slice(ic * 512, ic * 512 + 512)
                    nc.tensor.matmul(sum_ps[:, sl], lhsT=ones_mb[:],
                                     rhs=xb_sb[:, ct, sl],
                                     start=(ct == 0), stop=(ct == CT - 1))
                    nc.tensor.matmul(sq_ps[:, sl], lhsT=ones_mb[:], rhs=x2[:, sl],
                                     start=(ct == 0), stop=(ct == CT - 1))

            # mu (bf16 for the apply; bf16 is fine inside 512*mu^2 too)
            nc.scalar.activation(out=mu_b[:], in_=sum_ps[:], func=AF.Copy,
                                 scale=1.0 / CH)
            # 512*mu^2 ; (var+eps)*512 = (sq + 512*eps) - 512*mu^2
            nc.vector.tensor_tensor(out=m2_f[:], in0=mu_b[:], in1=sum_ps[:],
                                    op=OP.mult)
            nc.vector.scalar_tensor_tensor(out=ve_f[:], in0=sq_ps[:],
                                           scalar=float(CH * EPS), in1=m2_f[:],
                                           op0=OP.add, op1=OP.subtract)
            # rs = rsqrt(var+eps) = exp(-0.5*ln(var+eps)); keeps ACT on the
            # natural_log_exp table set for the entire kernel
            nc.scalar.activation(out=ve_f[:], in_=ve_f[:], func=AF.Ln,
                                 scale=1.0 / CH)
            nc.scalar.activation(out=rs_b[:], in_=ve_f[:], func=AF.Exp,
                                 scale=-0.5)

            for ct in range(CT):
                nc.vector.tensor_tensor(out=xn_sb[:, ct], in0=xb_sb[:, ct],
                                        in1=mu_b[:], op=OP.subtract)
                nc.vector.tensor_tensor(out=xn_sb[:, ct], in0=xn_sb[:, ct],
                                        in1=rs_b[:], op=OP.mult)

        # prefetch strips for the first head pair; residual x late (proj-only)
        strip_tiles = {}
        for h in (0, 1):
            st = strip_pool.tile([P, STRIP_W], BF16, name=f"strip{h}", tag="strip")
            nc.sync.dma_start(st[:], strips_d[h])
            strip_tiles[h] = st
        x_r = x_d.rearrange("(ct p) t -> p ct t", p=P)
        for ct in range(CT):
            nc.sync.dma_start(x_sb[:, ct], x_r[:, ct])

        # ---------- phase 2: Q, K, V projections ----------
        for dt in range(CT):
            dsl = slice(dt * P, dt * P + P)
            for ic in range(IC):
                sl = slice(ic * 512, ic * 512 + 512)
                q_ps = psB.tile([P, 512], F32, tag="small")
                for ck in range(CT):
                    nc.tensor.matmul(q_ps[:], lhsT=wq_sb[:, ck, dsl],
                                     rhs=xn_sb[:, ck, sl],
                                     start=(ck == 0), stop=(ck == CT - 1))
                nc.vector.tensor_scalar_add(out=qT_sb[:, dt, sl], in0=q_ps[:],
                                            scalar1=bqk_sb[:, 0, dt : dt + 1])
                k_ps = psB.tile([P, 512], F32, tag="small")
                for ck in range(CT):
                    nc.tensor.matmul(k_ps[:], lhsT=wk_sb[:, ck, dsl],
                                     rhs=xn_sb[:, ck, sl],
                                     start=(ck == 0), stop=(ck == CT - 1))
                nc.vector.tensor_scalar_add(out=kT_sb[:, dt, sl], in0=k_ps[:],
                                            scalar1=bqk_sb[:, 1, dt : dt + 1])

        for tt in range(TT):
            tsl = slice(tt * P, tt * P + P)
            v_ps = psB.tile([P, 512], F32, tag="small")
            for ck in range(CT):
                nc.tensor.matmul(v_ps[:], lhsT=xn_sb[:, ck, tsl],
                                 rhs=wv_sb[:, ck, :],
                                 start=(ck == 0), stop=False)
            nc.tensor.matmul(v_ps[:], lhsT=ones_rb[:, :P], rhs=brow_sb[:, 0, :],
                             start=False, stop=True)
            nc.vector.tensor_copy(
                out=v_view[:, tt, :, 0:HD],
                in_=v_ps[:].rearrange("p (h w) -> p h w", w=HD))

        # ---------- phase 3: attention, head pairs (A rows 0-63, B rows 64-127) --
        def zpath(h, o_ps):
            """replicate ln(Z) via K=1 matmul -> exp(-x) -> oTn."""
            for ic in range(IC):
                sl = slice(ic * 512, ic * 512 + 512)
                zl_ps = psB.tile([P, 512], F32, tag="small")
                nc.tensor.matmul(zl_ps[:HD, :], lhsT=ones16[HD : HD + 1, :],
                                 rhs=lnz_sb[HD : HD + 1, sl],
                                 start=True, stop=True)
                zrep = work.tile([HD, 512], F32, tag="zrep")
                nc.scalar.activation(out=zrep[:], in_=zl_ps[:HD, :], func=AF.Exp,
                                     scale=-1.0)
                nc.vector.tensor_tensor(out=oTn_sb[:, h, sl], in0=o_ps[:HD, sl],
                                        in1=zrep[:], op=OP.mult)

        # y_acc: residual + proj(heads 0-3), built mid-attention (pass 1)
        y_acc = singles.tile([P, CT, NT], F32)

        def proj_pass1(ct):
                csl = slice(ct * P, ct * P + P)
                for icc in range(IC):
                    sl = slice(icc * 512, icc * 512 + 512)
                    yp = psB.tile([P, 512], F32, tag="small",
                                  name=f"yp1_{ct}_{icc}")
                    for hh in range(4):
                        nc.tensor.matmul(yp[:], lhsT=wpP_sb[:, hh, csl],
                                         rhs=oTn_sb[:, hh, sl],
                                         start=(hh == 0), stop=False)
                    nc.tensor.matmul(yp[:], lhsT=brow_sb[:, 1, csl],
                                     rhs=ones_rb[:, :512],
                                     start=False, stop=True)
                    nc.vector.tensor_tensor(out=y_acc[:, ct, sl], in0=yp[:],
                                            in1=x_sb[:, ct, sl], op=OP.add)

        pending_z = None       # (head, o_ps) whose PE z-work is deferred
        for h in range(HEADS):
            dtl = h // 2
            drow = HD * (h % 2)
            strip = strip_tiles.pop(h)
            if h < HEADS - 2:
                st = strip_pool.tile([P, STRIP_W], BF16, name=f"strip{h + 2}",
                                     tag="strip")
                nc.sync.dma_start(st[:], strips_d[h + 2])
                strip_tiles[h + 2] = st

            at_tiles = []
            o_ps = None
            for jt in range(TT):
                s_ps = psA.tile([P, NT], F32, tag="big")
                off = (28 - 4 * jt) * 32
                for ic in range(IC):
                    sl = slice(ic * 512, ic * 512 + 512)
                    nc.tensor.matmul(
                        s_ps[:, sl],
                        lhsT=kT_sb[drow : drow + HD, dtl, jt * P : jt * P + P],
                        rhs=qT_sb[drow : drow + HD, dtl, sl],
                        start=True, stop=True)
                # exp(s) on ScalarE, then * exp(bias) on DVE (bf16 2x mode):
                # exp(s + b) = exp(s) * exp(b), strips hold exp(b) host-side
                eS = es_pool.tile([P, NT], BF16, name=f"eS_{h}_{jt}", tag="eS")
                nc.scalar.activation(out=eS[:], in_=s_ps[:], func=AF.Exp)
                aT = at_pool.tile([P, NT], BF16, name=f"aT_{h}_{jt}", tag="aT")
                nc.vector.tensor_tensor(out=aT[:], in0=eS[:],
                                        in1=strip[:, off : off + NT], op=OP.mult)
                at_tiles.append(aT)
                if jt == 0 and pending_z is not None:
                    # previous head's z-normalization matmuls, placed after
                    # this head's first score block so ln(Z) has drained
                    zpath(*pending_z)
                    pending_z = None
                if jt == 2 and h >= 4:
                    proj_pass1(h - 4)
                if jt > 1:
                    if jt == 2:
                        o_ps = ps_o.tile([HD + 1, NT], F32, tag="o",
                                         name=f"o_ps_{h}")
                    for ic in range(IC):
                        sl = slice(ic * 512, ic * 512 + 512)
                        nc.tensor.matmul(
                            o_ps[:, sl],
                            lhsT=v_sb[:, jt - 2, h * VW : h * VW + HD + 1],
                            rhs=at_tiles[jt - 2][:, sl],
                            start=(jt - 2 == 0), stop=False)
            for jl in (TT - 2, TT - 1):
                for ic in range(IC):
                    sl = slice(ic * 512, ic * 512 + 512)
                    nc.tensor.matmul(o_ps[:, sl],
                                     lhsT=v_sb[:, jl, h * VW : h * VW + HD + 1],
                                     rhs=at_tiles[jl][:, sl],
                                     start=False, stop=(jl == TT - 1))
            nc.scalar.activation(out=lnz_sb[HD : HD + 1, :],
                                 in_=o_ps[HD : HD + 1, :], func=AF.Ln)
            pending_z = (h, o_ps)
        zpath(*pending_z)

        # keep the PE array's HAM clock-gate warm across the z-path tail so
        # projection pass 2 runs at 2.4 GHz (results discarded)
        warm_ps = psA.tile([P, NT], F32, tag="big")
        for wi in range(8):
            nc.tensor.matmul(warm_ps[:, 0:512], lhsT=ones_mb[:],
                             rhs=xn_sb[:, 0, 0:512], start=True, stop=True)

        # ---------- phase 4: projection pass 2 (heads 4-7) + y_acc ----------
        for ct in range(CT):
            csl = slice(ct * P, ct * P + P)
            y_ps = [psB.tile([P, 512], F32, tag="small", name=f"y_ps_{ct}_{i}")
                    for i in range(IC)]
            for h in range(4, HEADS):
                for icc in range(IC):
                    sl = slice(icc * 512, icc * 512 + 512)
                    nc.tensor.matmul(y_ps[icc][:], lhsT=wpP_sb[:, h, csl],
                                     rhs=oTn_sb[:, h, sl],
                                     start=(h == 4), stop=(h == HEADS - 1))
            for icc in range(IC):
                sl = slice(icc * 512, icc * 512 + 512)
                yw = work.tile([P, 512], F32, tag="yw")
                nc.vector.tensor_tensor(out=yw[:], in0=y_ps[icc][:],
                                        in1=y_acc[:, ct, sl], op=OP.add)
                nc.sync.dma_start(y_d[csl, sl], yw[:])

    return nc


def _legalize_waits(nc, max_waits: int = 1):
    """Split multi-wait instructions into preceding same-engine NoOps.

    The TPB instruction encoding carries a single sync-wait slot and this
    walrus build refuses to legalize ("Too many sync wait commands"), so do
    it here: engines execute their queue in order, so a NoOp carrying one of
    the waits delays everything after it on that engine identically.
    """
    import orjson

    data = orjson.loads(mybir.module_to_json_bytes(nc.m))
    ctr = [0]

    def fix_block(block):
        out = []
        for inst in block.get("instructions", []):
            si = inst.get("sync_info") or {}
            waits = si.get("on_wait") or []
            if len(waits) > max_waits:
                for w in waits[max_waits:]:
                    ctr[0] += 1
                    nop = {
                        "name": f"I-WS{ctr[0]}",
                        "opcode": "NoOp",
                        "engine": inst["engine"],
                        "ins": [],
                        "outs": [],
                        "sync_info": {"on_wait": [w], "on_update": []},
                    }
                    if "debug" in inst:
                        nop["debug"] = inst["debug"]
                    out.append(nop)
                si = dict(si)
                si["on_wait"] = waits[:max_waits]
                inst["sync_info"] = si
            out.append(inst)
        block["instructions"] = out
        for b in block.get("blocks", []):
            fix_block(b)

    for fn in data["functions"]:
        for b in fn.get("blocks", []):
            fix_block(b)
    nc.m = mybir.module_from_json_bytes(orjson.dumps(data))
    return nc


_NC = None

BF = ml_dtypes.bfloat16


def _host_prep(x, norm_w, norm_b, wq, bq, wk, bk, wv, bv, wp, bp, rel):
    scale = HD ** -0.5
    # fold LN affine + score scale into the projection weights (exact algebra)
    wq_eff = (wq * norm_w[None, :]) * scale
    bq_eff = (bq + wq @ norm_b) * scale
    wk_eff = wk * norm_w[None, :]
    bk_eff = bk + wk @ norm_b
    wv_eff = wv * norm_w[None, :]
    bv_eff = bv + wv @ norm_b

    wqT = np.ascontiguousarray(wq_eff.T).astype(BF)
    wkT = np.ascontiguousarray(wk_eff.T).astype(BF)
    wvT = np.ascontiguousarray(wv_eff.T).astype(BF)
    # wp permuted so each head's 64 input rows sit at partitions 0..63
    wpP = np.ascontiguousarray(
        wp.T.reshape(HEADS, HD, CH).transpose(1, 0, 2)).astype(BF)

    bqk = np.stack([bq_eff, bk_eff]).astype(np.float32)
    brow = np.stack([bv_eff, bp]).astype(BF)
    strips = np.exp(_build_strips(np.asarray(rel, np.float32))).astype(BF)

    shared = {
        "wqT": wqT, "wkT": wkT, "wvT": wvT, "wpP": wpP,
        "bqk": bqk, "brow": brow, "strips": strips,
    }
    in_maps = []
    for b in range(B):
        m = dict(shared)
        xf = np.ascontiguousarray(x[b].reshape(CH, NT)).astype(np.float32)
        m["x"] = xf
        m["xb"] = xf.astype(BF)
        in_maps.append(m)
    return in_maps


def kernel(**inputs):
    global _NC
    if _NC is None:
        _NC = _legalize_waits(_build_nc())
    in_maps = _host_prep(**{k: np.asarray(v) for k, v in inputs.items()})
    res = run_bass_kernel_spmd(_NC, in_maps, list(range(B)))
    out = np.stack([res.results[b]["y"].reshape(CH, H, W) for b in range(B)])
    return out.astype(np.float32)


if __name__ == "__main__":
    nc = _build_nc()
    print("built OK")


# revision 15
# speedup vs baseline: 1.1947x; 1.0086x over previous
"""Trainium2 Bass kernel for nn_MHAAttention (LayerNorm2d + MHA w/ rel-pos bias + residual).

Sharding: data-parallel over batch - 8 batch elements, one per NeuronCore.
No collectives needed.  All matmuls run in bf16 (single-pass PE; fp32 would
double-pump LOW/HIGH), tolerance 2e-2 leaves ample headroom (measured ~4e-5).

Per-core pipeline:
  x (C=512 part-tiles, T=1024 free), LN stats via bf16 ones-matmuls;
  rsqrt(var+eps) = exp(-0.5*ln(var+eps)) so ONE ACT table set
  (natural_log_exp) serves the whole kernel - no table reloads.
  LN affine + 1/sqrt(d) score scale are folded into wq/wk/wv host-side.
  Q,K in (d part, t free); V in (t part, d free) per head [v(64) | 1 | pad],
  the ones column makes attn@V also produce the softmax denominator Z.
  Scores per head, transposed: sT[j,i] = k.q (K=64 matmul, 2x512 chunks).
  Rel-pos bias applied AFTER exp: aT = exp(s) * exp(bias) - the strips hold
  host-precomputed exp(bias) (block-Toeplitz sliding strip, bf16), so the
  bias add is a DVE bf16 2x-mode multiply instead of 128 extra matmuls.
  attn@V for each head is interleaved into the score stream with a 2-tile
  lag to keep the PE array busy (HAM clock-gate warm).  1/Z: ln(Z) on the
  single Z row, K=1 ones-matmul replication, exp(-x), DVE multiply.
  Each head's z-normalization matmuls are deferred into the next head's
  first score block so ln(Z) has drained.
  Projection is split: heads 0-3 are projected mid-attention (one channel
  tile per remaining head, residual x added there), heads 4-7 at the end;
  a short dummy-matmul burst keeps the PE clock-gate warm across the z-tail.
  Output staged through work tiles and DMA'd out per 512-column chunk.
"""

import sys

for _p in ("/opt/trn_rl_repo",):
    if _p not in sys.path:
        sys.path.insert(0, _p)

from contextlib import ExitStack

import numpy as np
import ml_dtypes

import concourse.bass as bass
import concourse.mybir as mybir
import concourse.tile as tile
from concourse.bass_utils import run_bass_kernel_spmd

F32 = mybir.dt.float32
BF16 = mybir.dt.bfloat16
F16 = mybir.dt.float16
AF = mybir.ActivationFunctionType
OP = mybir.AluOpType

B = 8
CH = 512
H = W = 32
NT = H * W          # 1024 tokens
HEADS = 8
HD = 64
EPS = 1e-6
P = 128
CT = CH // P        # 4 channel tiles
TT = NT // P        # 8 token tiles
IC = NT // 512      # 2 free-dim chunks of 512
STRIP_W = 60 * 32   # 1920
VW = 66             # per-head v stride: [v(64) | 1 | pad]


def _build_strips(rel: np.ndarray) -> np.ndarray:
    """(3969, 8) rel table -> (8, 128, 1920) bias strips.

    strip[h, 32*jh_l + jw, 32*g + iw] = T_h[g - jh_l + 3, iw - jw + 31]
    where T_h = rel[:, h].reshape(63, 63).
    bias.T block for key-tile jt is then strip[:, (28-4*jt)*32 : +1024].
    """
    T = rel.reshape(63, 63, HEADS)  # [a, b, h]
    jh_l = np.arange(4)[:, None, None, None]
    jw = np.arange(32)[None, :, None, None]
    g = np.arange(60)[None, None, :, None]
    iw = np.arange(32)[None, None, None, :]
    a = g - jh_l + 3          # in [0,62]
    b = iw - jw + 31          # in [0,62]
    a_b, b_b = np.broadcast_arrays(a, b)
    out = T[a_b, b_b, :]      # (4, 32, 60, 32, 8)
    out = np.ascontiguousarray(np.moveaxis(out, -1, 0)).reshape(HEADS, 128, STRIP_W)
    return out


def _build_nc() -> bass.Bass:
    nc = bass.Bass()

    x_d = nc.declare_dram_parameter("x", [CH, NT], F32, isOutput=False)
    xb_d = nc.declare_dram_parameter("xb", [CH, NT], BF16, isOutput=False)
    wqT_d = nc.declare_dram_parameter("wqT", [CH, CH], BF16, isOutput=False)
    wkT_d = nc.declare_dram_parameter("wkT", [CH, CH], BF16, isOutput=False)
    wvT_d = nc.declare_dram_parameter("wvT", [CH, CH], BF16, isOutput=False)
    wpP_d = nc.declare_dram_parameter("wpP", [HD, HEADS, CH], BF16, isOutput=False)
    bqk_d = nc.declare_dram_parameter("bqk", [2, CH], F32, isOutput=False)
    brow_d = nc.declare_dram_parameter("brow", [2, CH], BF16, isOutput=False)
    strips_d = nc.declare_dram_parameter("strips", [HEADS, P, STRIP_W], BF16,
                                         isOutput=False)
    y_d = nc.declare_dram_parameter("y", [CH, NT], F32, isOutput=True)

    with tile.TileContext(nc) as tc, ExitStack() as ctx:
        singles = ctx.enter_context(tc.tile_pool(name="singles", bufs=1))
        work = ctx.enter_context(tc.tile_pool(name="work", bufs=4))
        es_pool = ctx.enter_context(tc.tile_pool(name="es_pool", bufs=3))
        strip_pool = ctx.enter_context(tc.tile_pool(name="strip_pool", bufs=4))
        at_pool = ctx.enter_context(tc.tile_pool(name="at_pool", bufs=8))
        # PSUM budget (8 banks): psA (128,1024)x3bufs = 6 banks (everything
        # except the attn@V accumulator); ps_o 2 banks.
        psA = ctx.enter_context(tc.tile_pool(name="psA", bufs=2, space="PSUM"))
        psB = ctx.enter_context(tc.tile_pool(name="psB", bufs=2, space="PSUM"))
        ps_o = ctx.enter_context(tc.tile_pool(name="ps_o", bufs=1, space="PSUM"))

        # ---------- persistent SBUF ----------
        x_sb = singles.tile([P, CT, NT], F32)        # residual source
        xb_sb = singles.tile([P, CT, NT], BF16)      # bf16 x for stats
        xn_sb = singles.tile([P, CT, NT], BF16)      # LN output
        qT_sb = singles.tile([P, CT, NT], BF16)      # (d part, t free)
        kT_sb = singles.tile([P, CT, NT], BF16)
        v_sb = singles.tile([P, TT, HEADS * VW], BF16)
        oTn_sb = singles.tile([HD, HEADS, NT], BF16)  # normalized per-head oT

        wq_sb = singles.tile([P, CT, CH], BF16)
        wk_sb = singles.tile([P, CT, CH], BF16)
        wv_sb = singles.tile([P, CT, CH], BF16)
        wpP_sb = singles.tile([HD, HEADS, CH], BF16)
        bqk_sb = singles.tile([P, 2, CT], F32)       # per-partition bias for q,k
        brow_sb = singles.tile([1, 2, CH], BF16)     # bv_eff, bp rows
        ones_mb = singles.tile([P, P], BF16)         # bf16 ones (LN stats lhsT)
        ones_rb = singles.tile([1, 512], BF16)       # bf16 ones row
        ones16 = singles.tile([HD + 1, HD], F16)     # f16 ones (zrep lhsT, row 64)
        lnz_sb = singles.tile([HD + 1, NT], F16)     # ln(Z) row at partition 64

        mu_b = singles.tile([P, NT], BF16)
        rs_b = singles.tile([P, NT], BF16)
        m2_f = singles.tile([P, NT], F32)
        ve_f = singles.tile([P, NT], F32)

        nc.vector.memset(ones_mb[:], 1.0)
        nc.vector.memset(ones_rb[:], 1.0)
        nc.vector.memset(ones16[:], 1.0)
        # per-ct x chunks so LN stats can start on the first chunk;
        # q weights early so the QKV phase is not gated on DMA
        xb_r = xb_d.rearrange("(ct p) t -> p ct t", p=P)
        for ct in range(CT):
            nc.sync.dma_start(xb_sb[:, ct], xb_r[:, ct])
        nc.sync.dma_start(bqk_sb[:], bqk_d.rearrange("i (o p) -> p i o", p=P))
        nc.sync.dma_start(brow_sb[:], brow_d[None, :, :])
        nc.sync.dma_start(wq_sb[:], wqT_d.rearrange("(ck p) d -> p ck d", p=P))
        nc.sync.dma_start(wk_sb[:], wkT_d.rearrange("(ck p) d -> p ck d", p=P))
        nc.sync.dma_start(wv_sb[:], wvT_d.rearrange("(ck p) d -> p ck d", p=P))
        nc.sync.dma_start(wpP_sb[:], wpP_d[:])

        # ones columns of v
        v_view = v_sb[:].rearrange("p tt (h w) -> p tt h w", w=VW)
        nc.vector.memset(v_view[:, :, :, HD : HD + 1], 1.0)

        # ---------- phase 1: LayerNorm ----------
        with tc.tile_pool(name="ln_pool", bufs=2) as lnp:
            sum_ps = psA.tile([P, NT], F32, tag="big")
            sq_ps = psA.tile([P, NT], F32, tag="big")
            for ct in range(CT):
                x2 = lnp.tile([P, NT], BF16, name=f"x2_{ct}", tag="x2")
                nc.vector.tensor_tensor(out=x2[:], in0=xb_sb[:, ct],
                                        in1=xb_sb[:, ct], op=OP.mult)
                for ic in range(IC):
                    sl = slice(ic * 512, ic * 512 + 512)
                    nc.tensor.matmul(sum_ps[:, sl], lhsT=ones_mb[:],
                                     rhs=xb_sb[:, ct, sl],
                                     start=(ct == 0), stop=(ct == CT - 1))
                    nc.tensor.matmul(sq_ps[:, sl], lhsT=ones_mb[:], rhs=x2[:, sl],
                                     start=(ct == 0), stop=(ct == CT - 1))

            # mu (bf16 for the apply; bf16 is fine inside 512*mu^2 too)
            nc.scalar.activation(out=mu_b[:], in_=sum_ps[:], func=AF.Copy,
                                 scale=1.0 / CH)
            # 512*mu^2 ; (var+eps)*512 = (sq + 512*eps) - 512*mu^2
            nc.vector.tensor_tensor(out=m2_f[:], in0=mu_b[:], in1=sum_ps[:],
                                    op=OP.mult)
            nc.vector.scalar_tensor_tensor(out=ve_f[:], in0=sq_ps[:],
                                           scalar=float(CH * EPS), in1=m2_f[:],
                                           op0=OP.add, op1=OP.subtract)
            # rs = rsqrt(var+eps) = exp(-0.5*ln(var+eps)); keeps ACT on the
            # natural_log_exp table set for the entire kernel
            nc.scalar.activation(out=ve_f[:], in_=ve_f[:], func=AF.Ln,
                                 scale=1.0 / CH)
            nc.scalar.activation(out=rs_b[:], in_=ve_f[:], func=AF.Exp,
                                 scale=-0.5)

            for ct in range(CT):
                nc.vector.tensor_tensor(out=xn_sb[:, ct], in0=xb_sb[:, ct],
                                        in1=mu_b[:], op=OP.subtract)
                nc.vector.tensor_tensor(out=xn_sb[:, ct], in0=xn_sb[:, ct],
                                        in1=rs_b[:], op=OP.mult)

        # prefetch strips for the first head pair; residual x late (proj-only)
        strip_tiles = {}
        for h in (0, 1):
            st = strip_pool.tile([P, STRIP_W], BF16, name=f"strip{h}", tag="strip")
            nc.sync.dma_start(st[:], strips_d[h])
            strip_tiles[h] = st
        x_r = x_d.rearrange("(ct p) t -> p ct t", p=P)
        for ct in range(CT):
            nc.sync.dma_start(x_sb[:, ct], x_r[:, ct])

        # ---------- phase 2: Q, K, V projections ----------
        for dt in range(CT):
            dsl = slice(dt * P, dt * P + P)
            for ic in range(IC):
                sl = slice(ic * 512, ic * 512 + 512)
                q_ps = psB.tile([P, 512], F32, tag="small")
                for ck in range(CT):
                    nc.tensor.matmul(q_ps[:], lhsT=wq_sb[:, ck, dsl],
                                     rhs=xn_sb[:, ck, sl],
                                     start=(ck == 0), stop=(ck == CT - 1))
                nc.vector.tensor_scalar_add(out=qT_sb[:, dt, sl], in0=q_ps[:],
                                            scalar1=bqk_sb[:, 0, dt : dt + 1])
                k_ps = psB.tile([P, 512], F32, tag="small")
                for ck in range(CT):
                    nc.tensor.matmul(k_ps[:], lhsT=wk_sb[:, ck, dsl],
                                     rhs=xn_sb[:, ck, sl],
                                     start=(ck == 0), stop=(ck == CT - 1))
                nc.vector.tensor_scalar_add(out=kT_sb[:, dt, sl], in0=k_ps[:],
                                            scalar1=bqk_sb[:, 1, dt : dt + 1])

        for tt in range(TT):
            tsl = slice(tt * P, tt * P + P)
            v_ps = psB.tile([P, 512], F32, tag="small")
            for ck in range(CT):
                nc.tensor.matmul(v_ps[:], lhsT=xn_sb[:, ck, tsl],
                                 rhs=wv_sb[:, ck, :],
                                 start=(ck == 0), stop=False)
            nc.tensor.matmul(v_ps[:], lhsT=ones_rb[:, :P], rhs=brow_sb[:, 0, :],
                             start=False, stop=True)
            nc.vector.tensor_copy(
                out=v_view[:, tt, :, 0:HD],
                in_=v_ps[:].rearrange("p (h w) -> p h w", w=HD))

        # ---------- phase 3: attention, head pairs (A rows 0-63, B rows 64-127) --
        def zpath(h, o_ps):
            """replicate ln(Z) via K=1 matmul -> exp(-x) -> oTn."""
            for ic in range(IC):
                sl = slice(ic * 512, ic * 512 + 512)
                zl_ps = psB.tile([P, 512], F32, tag="small")
                nc.tensor.matmul(zl_ps[:HD, :], lhsT=ones16[HD : HD + 1, :],
                                 rhs=lnz_sb[HD : HD + 1, sl],
                                 start=True, stop=True)
                zrep = work.tile([HD, 512], F32, tag="zrep")
                nc.scalar.activation(out=zrep[:], in_=zl_ps[:HD, :], func=AF.Exp,
                                     scale=-1.0)
                nc.vector.tensor_tensor(out=oTn_sb[:, h, sl], in0=o_ps[:HD, sl],
                                        in1=zrep[:], op=OP.mult)

        # y_acc: residual + proj(heads 0-3), built mid-attention (pass 1)
        y_acc = singles.tile([P, CT, NT], F32)

        def proj_pass1(ct):
                csl = slice(ct * P, ct * P + P)
                for icc in range(IC):
                    sl = slice(icc * 512, icc * 512 + 512)
                    yp = psB.tile([P, 512], F32, tag="small",
                                  name=f"yp1_{ct}_{icc}")
                    for hh in range(4):
                        nc.tensor.matmul(yp[:], lhsT=wpP_sb[:, hh, csl],
                                         rhs=oTn_sb[:, hh, sl],
                                         start=(hh == 0), stop=False)
                    nc.tensor.matmul(yp[:], lhsT=brow_sb[:, 1, csl],
                                     rhs=ones_rb[:, :512],
                                     start=False, stop=True)
                    nc.vector.tensor_tensor(out=y_acc[:, ct, sl], in0=yp[:],
                                            in1=x_sb[:, ct, sl], op=OP.add)

        pending_z = None       # (head, o_ps) whose PE z-work is deferred
        for h in range(HEADS):
            dtl = h // 2
            drow = HD * (h % 2)
            strip = strip_tiles.pop(h)
            if h < HEADS - 2:
                st = strip_pool.tile([P, STRIP_W], BF16, name=f"strip{h + 2}",
                                     tag="strip")
                nc.sync.dma_start(st[:], strips_d[h + 2])
                strip_tiles[h + 2] = st

            at_tiles = []
            o_ps = None
            for jt in range(TT):
                s_ps = psA.tile([P, NT], F32, tag="big")
                off = (28 - 4 * jt) * 32
                for ic in range(IC):
                    sl = slice(ic * 512, ic * 512 + 512)
                    nc.tensor.matmul(
                        s_ps[:, sl],
                        lhsT=kT_sb[drow : drow + HD, dtl, jt * P : jt * P + P],
                        rhs=qT_sb[drow : drow + HD, dtl, sl],
                        start=True, stop=True)
                # exp(s) on ScalarE, then * exp(bias) on DVE (bf16 2x mode):
                # exp(s + b) = exp(s) * exp(b), strips hold exp(b) host-side
                eS = es_pool.tile([P, NT], BF16, name=f"eS_{h}_{jt}", tag="eS")
                nc.scalar.activation(out=eS[:], in_=s_ps[:], func=AF.Exp)
                aT = at_pool.tile([P, NT], BF16, name=f"aT_{h}_{jt}", tag="aT")
                nc.vector.tensor_tensor(out=aT[:], in0=eS[:],
                                        in1=strip[:, off : off + NT], op=OP.mult)
                at_tiles.append(aT)
                if jt == 0 and pending_z is not None:
                    # previous head's z-normalization matmuls, placed after
                    # this head's first score block so ln(Z) has drained
                    zpath(*pending_z)
                    pending_z = None
                if jt == 2 and h >= 4:
                    proj_pass1(h - 4)
                if jt > 1:
                    if jt == 2:
                        o_ps = ps_o.tile([HD + 1, NT], F32, tag="o",
                                         name=f"o_ps_{h}")
                    for ic in range(IC):
                        sl = slice(ic * 512, ic * 512 + 512)
                        nc.tensor.matmul(
                            o_ps[:, sl],
                            lhsT=v_sb[:, jt - 2, h * VW : h * VW + HD + 1],
                            rhs=at_tiles[jt - 2][:, sl],
                            start=(jt - 2 == 0), stop=False)
            for jl in (TT - 2, TT - 1):
                for ic in range(IC):
                    sl = slice(ic * 512, ic * 512 + 512)
                    nc.tensor.matmul(o_ps[:, sl],
                                     lhsT=v_sb[:, jl, h * VW : h * VW + HD + 1],
                                     rhs=at_tiles[jl][:, sl],
                                     start=False, stop=(jl == TT - 1))
            nc.scalar.activation(out=lnz_sb[HD : HD + 1, :],
                                 in_=o_ps[HD : HD + 1, :], func=AF.Ln)
            pending_z = (h, o_ps)
        zpath(*pending_z)

        # keep the PE array's HAM clock-gate warm across the z-path tail so
        # projection pass 2 runs at 2.4 GHz (results discarded)
        warm_ps = psA.tile([P, NT], F32, tag="big")
        for wi in range(8):
            nc.tensor.matmul(warm_ps[:, 0:512], lhsT=ones_mb[:],
                             rhs=xn_sb[:, 0, 0:512], start=True, stop=True)

        # ---------- phase 4: projection pass 2 (heads 4-7) + y_acc ----------
        for ct in range(CT):
            csl = slice(ct * P, ct * P + P)
            y_ps = [psB.tile([P, 512], F32, tag="small", name=f"y_ps_{ct}_{i}")
                    for i in range(IC)]
            for h in range(4, HEADS):
                for icc in range(IC):
                    sl = slice(icc * 512, icc * 512 + 512)
                    nc.tensor.matmul(y_ps[icc][:], lhsT=wpP_sb[:, h, csl],
                                     rhs=oTn_sb[:, h, sl],
                                     start=(h == 4), stop=(h == HEADS - 1))
            for icc in range(IC):
                sl = slice(icc * 512, icc * 512 + 512)
                yw = work.tile([P, 512], F32, tag="yw")
                nc.vector.tensor_tensor(out=yw[:], in0=y_ps[icc][:],
                                        in1=y_acc[:, ct, sl], op=OP.add)
                nc.sync.dma_start(y_d[csl, sl], yw[:])

    return nc


def _legalize_waits(nc, max_waits: int = 1):
    """Split multi-wait instructions into preceding same-engine NoOps.

    The TPB instruction encoding carries a single sync-wait slot and this
    walrus build refuses to legalize ("Too many sync wait commands"), so do
    it here: engines execute their queue in order, so a NoOp carrying one of
    the waits delays everything after it on that engine identically.
    """
    import orjson

    data = orjson.loads(mybir.module_to_json_bytes(nc.m))
    ctr = [0]

    def fix_block(block):
        out = []
        for inst in block.get("instructions", []):
            si = inst.get("sync_info") or {}
            waits = si.get("on_wait") or []
            if len(waits) > max_waits:
                for w in waits[max_waits:]:
                    ctr[0] += 1
                    nop = {
                        "name": f"I-WS{ctr[0]}",
                        "opcode": "NoOp",
                        "engine": inst["engine"],
                        "ins": [],
                        "outs": [],
                        "sync_info": {"on_wait": [w], "on_update": []},
                    }
                    if "debug" in inst:
                        nop["debug"] = inst["debug"]
                    out.append(nop)
                si = dict(si)
                si["on_wait"] = waits[:max_waits]
                inst["sync_info"] = si
            out.append(inst)
        block["instructions"] = out
        for b in block.get("blocks", []):
            fix_block(b)

    for fn in data["functions"]:
        for b in fn.get("blocks", []):
            fix_block(b)
    nc.m = mybir.module_from_json_bytes(orjson.dumps(data))
    return nc


_NC = None

BF = ml_dtypes.bfloat16


def _host_prep(x, norm_w, norm_b, wq, bq, wk, bk, wv, bv, wp, bp, rel):
    scale = HD ** -0.5
    # fold LN affine + score scale into the projection weights (exact algebra)
    wq_eff = (wq * norm_w[None, :]) * scale
    bq_eff = (bq + wq @ norm_b) * scale
    wk_eff = wk * norm_w[None, :]
    bk_eff = bk + wk @ norm_b
    wv_eff = wv * norm_w[None, :]
    bv_eff = bv + wv @ norm_b

    wqT = np.ascontiguousarray(wq_eff.T).astype(BF)
    wkT = np.ascontiguousarray(wk_eff.T).astype(BF)
    wvT = np.ascontiguousarray(wv_eff.T).astype(BF)
    # wp permuted so each head's 64 input rows sit at partitions 0..63
    wpP = np.ascontiguousarray(
        wp.T.reshape(HEADS, HD, CH).transpose(1, 0, 2)).astype(BF)

    bqk = np.stack([bq_eff, bk_eff]).astype(np.float32)
    brow = np.stack([bv_eff, bp]).astype(BF)
    strips = np.exp(_build_strips(np.asarray(rel, np.float32))).astype(BF)

    shared = {
        "wqT": wqT, "wkT": wkT, "wvT": wvT, "wpP": wpP,
        "bqk": bqk, "brow": brow, "strips": strips,
    }
    in_maps = []
    for b in range(B):
        m = dict(shared)
        xf = np.ascontiguousarray(x[b].reshape(CH, NT)).astype(np.float32)
        m["x"] = xf
        m["xb"] = xf.astype(BF)
        in_maps.append(m)
    return in_maps


def kernel(**inputs):
    global _NC
    if _NC is None:
        _NC = _legalize_waits(_build_nc())
    in_maps = _host_prep(**{k: np.asarray(v) for k, v in inputs.items()})
    res = run_bass_kernel_spmd(_NC, in_maps, list(range(B)))
    out = np.stack([res.results[b]["y"].reshape(CH, H, W) for b in range(B)])
    return out.astype(np.float32)


if __name__ == "__main__":
    nc = _build_nc()
    print("built OK")
